# revision 1
# baseline (speedup 1.0000x reference)
"""Trainium2 Bass kernel for ConditionalTriangleAttention.

Reference computation (B=2, N=40, HID=256, NH=8, CD=128, HD=32):
  cf = edge_features * sigmoid(cond@Wcg+bcg) + (cond@Wcp+bcp)     (per batch)
  Q/K/V = cf @ W_{q,k,v} + b                                       [B,N,N,NH,HD]
  scores = einsum('bijhd,bklhd->bijklh', Q, K)/sqrt(HD) + bias     (bias const over k,l)
  attn = softmax over l;  attended = einsum('bijklh,bklhd->bijhd', attn, V)
  out = (attended * sigmoid(attended@Wtg+btg)) @ Wo + bo

With edge_mask all-ones (guaranteed by the input spec) the additive bias is
constant along the softmax axis and cancels, so Wtb/btb/edge_mask are no-ops.
A numpy fallback handles any other mask.

Sharding: 8 cores, each owns 400 query rows (b = core//4, i-rows slice) and
computes all heads for them end-to-end -- no collectives.  Per core:
  - transpose ef to efT [hid, kl] on PE, fold the conditional gate into the
    projection weights (W' = diag(g)W, b' = p@W + b)
  - S^T[kl, q] per head via row-tiled matmuls (K=32, 4 heads concurrently)
  - exp on ACT straight out of PSUM (scale=1/sqrt(HD) folded into exp)
  - softmax denominators via segment-indicator matmul, reciprocal on DVE
  - 1/sums broadcast l-wise via a DRAM-bounced DMA expansion
  - P^T = E^T * R on DVE (bf16, 2x mode);  U^T = V.T-contraction col-tiled
  - phase 3 (gate + out proj) locally per q-slice
"""

import os
import sys

for _p in ("/opt/trn_rl_repo", "/root/.axon_site/_ro/trn_rl_repo"):
    if os.path.isdir(_p) and _p not in sys.path:
        sys.path.insert(0, _p)

import numpy as np

B, N, HID, NH, CD = 2, 40, 256, 8, 128
HD = HID // NH            # 32
KL = N * N                # 1600
NQ = KL // 4              # 400 query rows per core
NCORES = 8
ALPHA = 1.0 / np.sqrt(np.float32(HD))

CHUNK = 120               # kl chunk: 3 k-groups of 40 -> uniform R-expansion AP
NCH = 14                  # 13*120 + 40
LAST_P = KL - (NCH - 1) * CHUNK   # 40

_COMPILED = None


def _chunk_p(ck):
    return CHUNK if ck < NCH - 1 else LAST_P


def _build_nc(stage=None):
    if stage is None:
        stage = int(os.environ.get("KSTAGE", "4"))
    import concourse.bass as bass
    import concourse.tile as tile
    from concourse import bacc, mybir
    from concourse.masks import make_identity

    FP = mybir.dt.float32
    BF = mybir.dt.bfloat16
    AF = mybir.ActivationFunctionType
    OP = mybir.AluOpType

    nc = bacc.Bacc(None, target_bir_lowering=False)

    ef = nc.dram_tensor("ef", [KL, HID], FP, kind="ExternalInput")
    efq = nc.dram_tensor("efq", [NQ, HID], FP, kind="ExternalInput")
    cond = nc.dram_tensor("cond", [1, CD], FP, kind="ExternalInput")
    Wq = nc.dram_tensor("Wq", [HID, HID], FP, kind="ExternalInput")
    Wk = nc.dram_tensor("Wk", [HID, HID], FP, kind="ExternalInput")
    Wv = nc.dram_tensor("Wv", [HID, HID], FP, kind="ExternalInput")
    Wtg = nc.dram_tensor("Wtg", [HID, HID], FP, kind="ExternalInput")
    Wo = nc.dram_tensor("Wo", [HID, HID], FP, kind="ExternalInput")
    Wcp = nc.dram_tensor("Wcp", [CD, HID], FP, kind="ExternalInput")
    Wcg = nc.dram_tensor("Wcg", [CD, HID], FP, kind="ExternalInput")
    bq = nc.dram_tensor("bq", [1, HID], FP, kind="ExternalInput")
    bk = nc.dram_tensor("bk", [1, HID], FP, kind="ExternalInput")
    bv = nc.dram_tensor("bv", [1, HID], FP, kind="ExternalInput")
    btg = nc.dram_tensor("btg", [1, HID], FP, kind="ExternalInput")
    bo = nc.dram_tensor("bo", [1, HID], FP, kind="ExternalInput")
    bcp = nc.dram_tensor("bcp", [1, HID], FP, kind="ExternalInput")
    bcg = nc.dram_tensor("bcg", [1, HID], FP, kind="ExternalInput")
    out = nc.dram_tensor("out", [NQ, HID], FP, kind="ExternalOutput")

    # DRAM bounce for the 1/sums l-broadcast: [quad][part 128][pair 2][q 400]
    r_dram = nc.dram_tensor("rbounce", [2, 128, 2, NQ], BF, kind="Internal")

    with tile.TileContext(nc) as tc:
        with tc.tile_pool(name="persist", bufs=1) as sb:
            ident = sb.tile([128, 128], FP, tag="ident")
            make_identity(nc, ident)
            ones11 = sb.tile([1, 1], FP, tag="ones11")
            nc.vector.memset(ones11, 1.0)
            onesq = sb.tile([1, NQ], FP, tag="onesq")
            nc.vector.memset(onesq, 1.0)
            ones128 = sb.tile([1, 128], FP, tag="ones128")
            nc.vector.memset(ones128, 1.0)
            onescol_bf = sb.tile([1, CHUNK], BF, tag="onescol")
            nc.vector.memset(onescol_bf, 1.0)
            zrow = sb.tile([1, 128], FP, tag="zrow")
            nc.vector.memset(zrow, 0.0)

            # Segment indicator: iseg[p, 39 + p//40] = 1, zero elsewhere.
            # lhsT for chunk ck is iseg[:, 39-3ck : 39-3ck+64].
            iseg = sb.tile([CHUNK, 104], BF, tag="iseg")
            nc.vector.memset(iseg, 1.0)
            # keep 1 where j <= 39 + p//40  <=>  p - 40j + 1560 >= 0
            nc.gpsimd.affine_select(out=iseg[:], in_=iseg[:],
                                    pattern=[[-40, 104]],
                                    compare_op=mybir.AluOpType.is_ge,
                                    fill=0.0, base=1560, channel_multiplier=1)
            # keep 1 where j >= 39 + p//40  <=>  40j - p - 1521 >= 0
            nc.gpsimd.affine_select(out=iseg[:], in_=iseg[:],
                                    pattern=[[40, 104]],
                                    compare_op=mybir.AluOpType.is_ge,
                                    fill=0.0, base=-1521, channel_multiplier=-1)

            # ---- weight / input staging (fp32) ----
            w_sb = {}
            for name, t, pdim in (("Wq", Wq, HID), ("Wk", Wk, HID), ("Wv", Wv, HID),
                                  ("Wtg", Wtg, HID), ("Wo", Wo, HID),
                                  ("Wcp", Wcp, CD), ("Wcg", Wcg, CD)):
                nch = pdim // 128
                tl = sb.tile([128, nch, HID], FP, tag="w_" + name)
                for m in range(nch):
                    nc.sync.dma_start(out=tl[:, m, :], in_=t[128 * m:128 * (m + 1), :])
                w_sb[name] = tl
            b_sb = {}
            for name, t in (("bq", bq), ("bk", bk), ("bv", bv), ("btg", btg),
                            ("bo", bo), ("bcp", bcp), ("bcg", bcg)):
                tl = sb.tile([1, HID], FP, tag="b_" + name)
                nc.sync.dma_start(out=tl[:], in_=t[:])
                b_sb[name] = tl
            cond_sb = sb.tile([1, CD], FP, tag="cond_sb")
            nc.sync.dma_start(out=cond_sb[:], in_=cond[:])

            # persistent bf16 operands
            efT = sb.tile([128, 2, KL], BF, tag="efT")
            efqT = sb.tile([128, 2, NQ], BF, tag="efqT")
            KT = sb.tile([128, 2, KL], BF, tag="KT")
            QT = sb.tile([128, 2, NQ], BF, tag="QT")
            Vt = sb.tile([128, NCH, HID], BF, tag="Vt")
            ET0 = sb.tile([CHUNK, 4, NCH, NQ], BF, tag="ET0")
            ET1 = sb.tile([CHUNK, 4, NCH, NQ], BF, tag="ET1")
            PT = sb.tile([CHUNK, 4, NCH, NQ], BF, tag="PT")
            ETs = (ET0, ET1)
            attT = sb.tile([128, 2, NQ], BF, tag="attT")
            gateT = sb.tile([128, 2, NQ], BF, tag="gateT")
            gatedT = sb.tile([128, 2, NQ], BF, tag="gatedT")
            rq = sb.tile([128, 2, 2, NQ], BF, tag="rq")      # [quad][? pairs]
            # gated projection weights (bf16)
            Wqp = sb.tile([128, 2, HID], BF, tag="Wqp")
            Wkp = sb.tile([128, 2, HID], BF, tag="Wkp")
            Wvp = sb.tile([128, 2, HID], BF, tag="Wvp")
            Wtgb = sb.tile([128, 2, HID], BF, tag="Wtgb")
            Wob = sb.tile([128, 2, HID], BF, tag="Wob")
            # per-partition columns
            gT = sb.tile([128, 2, 1], FP, tag="gT")
            pT = sb.tile([128, 2, 1], FP, tag="pT")
            bqT = sb.tile([128, 2, 1], FP, tag="bqT")
            bkT = sb.tile([128, 2, 1], FP, tag="bkT")
            bvrow = sb.tile([1, HID], BF, tag="bvrow")

            # =============== phase 1 ===============
            with tc.tile_pool(name="p1psum", bufs=2, space="PSUM") as pp, \
                 tc.tile_pool(name="p1psumB", bufs=2, space="PSUM") as ppB, \
                 tc.tile_pool(name="efstage", bufs=4) as efst:

                # condT column [128,1]
                ct_ps = pp.tile([128, 1], FP, tag="tiny")
                nc.tensor.matmul(out=ct_ps[:], lhsT=cond_sb[:], rhs=ones11[:],
                                 start=True, stop=True)
                condT = sb.tile([128, 1], FP, tag="condT")
                nc.vector.tensor_copy(condT[:], ct_ps[:])

                # gates / proj columns per hid chunk
                for m in range(2):
                    gp_ps = pp.tile([128, 1], FP, tag="tiny")
                    nc.tensor.matmul(out=gp_ps[:], lhsT=w_sb["Wcg"][:, 0, 128 * m:128 * (m + 1)],
                                     rhs=condT[:], start=True, stop=False)
                    nc.tensor.matmul(out=gp_ps[:], lhsT=b_sb["bcg"][:, 128 * m:128 * (m + 1)],
                                     rhs=ones11[:], start=False, stop=True)
                    # sigmoid(x) = 1/(1+exp(-x))
                    nc.scalar.activation(out=gT[:, m, :], in_=gp_ps[:], func=AF.Exp,
                                         scale=-1.0)
                    nc.vector.tensor_scalar_add(gT[:, m, :], gT[:, m, :], 1.0)
                    nc.vector.reciprocal(gT[:, m, :], gT[:, m, :])

                    pp_ps = pp.tile([128, 1], FP, tag="tiny")
                    nc.tensor.matmul(out=pp_ps[:], lhsT=w_sb["Wcp"][:, 0, 128 * m:128 * (m + 1)],
                                     rhs=condT[:], start=True, stop=False)
                    nc.tensor.matmul(out=pp_ps[:], lhsT=b_sb["bcp"][:, 128 * m:128 * (m + 1)],
                                     rhs=ones11[:], start=False, stop=True)
                    nc.vector.tensor_copy(pT[:, m, :], pp_ps[:])

                # gated weights W' = diag(g) W  (bf16), per input-chunk m
                for (wn, dst) in (("Wq", Wqp), ("Wk", Wkp), ("Wv", Wvp)):
                    for m in range(2):
                        nc.vector.tensor_scalar_mul(dst[:, m, :], w_sb[wn][:, m, :],
                                                    gT[:, m, :])
                for (wn, dst) in (("Wtg", Wtgb), ("Wo", Wob)):
                    for m in range(2):
                        nc.vector.tensor_copy(dst[:, m, :], w_sb[wn][:, m, :])

                # bias columns b' = (p @ W + b)^T  for q,k
                for (wn, bn, dst) in (("Wq", "bq", bqT), ("Wk", "bk", bkT)):
                    for m in range(2):
                        bps = pp.tile([128, 1], FP, tag="tiny")
                        for c in range(2):
                            nc.tensor.matmul(out=bps[:],
                                             lhsT=w_sb[wn][:, c, 128 * m:128 * (m + 1)],
                                             rhs=pT[:, c, :], start=(c == 0), stop=False)
                        nc.tensor.matmul(out=bps[:], lhsT=b_sb[bn][:, 128 * m:128 * (m + 1)],
                                         rhs=ones11[:], start=False, stop=True)
                        nc.vector.tensor_copy(dst[:, m, :], bps[:])
                # bv' as a row (used via rank-1 matmul into V)
                bvr_ps = pp.tile([1, HID], FP, tag="tiny")
                for c in range(2):
                    nc.tensor.matmul(out=bvr_ps[:], lhsT=pT[:, c, :],
                                     rhs=w_sb["Wv"][:, c, :], start=(c == 0), stop=False)
                nc.tensor.matmul(out=bvr_ps[:], lhsT=ones11[:], rhs=b_sb["bv"][:],
                                 start=False, stop=True)
                nc.vector.tensor_copy(bvrow[:], bvr_ps[:])

                # ---- transpose ef and efq into efT / efqT (bf16) ----
                def do_transpose(src_dram, nrows, dstT, ncols_total):
                    ntile = (nrows + 127) // 128
                    for qt in range(ntile):
                        r0 = qt * 128
                        rn = min(128, nrows - r0)
                        stg = efst.tile([128, HID], FP, tag="efstg")
                        nc.sync.dma_start(out=stg[0:rn, :], in_=src_dram[r0:r0 + rn, :])
                        for m in range(2):
                            tp = ppB.tile([128, 128], FP, tag="tp")
                            nc.tensor.transpose(out=tp[:, 0:rn],
                                                in_=stg[0:rn, 128 * m:128 * (m + 1)],
                                                identity=ident[0:rn, 0:rn])
                            nc.vector.tensor_copy(dstT[:, m, r0:r0 + rn], tp[:, 0:rn])

                do_transpose(ef, KL, efT, KL)
                do_transpose(efq, NQ, efqT, NQ)

            if stage < 2:
                pass
            # =============== projections ===============
            with tc.tile_pool(name="kpsum", bufs=1, space="PSUM") as kpp, \
                 tc.tile_pool(name="projpsum", bufs=2, space="PSUM") as prp, \
                 tc.tile_pool(name="vpsum", bufs=2, space="PSUM") as vpp:
                # K^T (all heads) [hid', kl], + bias via tensor_scalar on evac
                for m in range(2):
                    kps = kpp.tile([128, 2048], FP, tag="kps")
                    for nb, (o, w) in enumerate(((0, 512), (512, 512), (1024, 512), (1536, 64))):
                        for c in range(2):
                            nc.tensor.matmul(out=kps[:, o:o + w],
                                             lhsT=Wkp[:, c, 128 * m:128 * (m + 1)],
                                             rhs=efT[:, c, o:o + w],
                                             start=(c == 0), stop=(c == 1))
                    nc.vector.tensor_scalar_add(KT[:, m, :], kps[:, 0:KL], bkT[:, m, :])
                # Q^T from efqT
                for m in range(2):
                    qps = prp.tile([128, 512], FP, tag="qps")
                    for c in range(2):
                        nc.tensor.matmul(out=qps[:, 0:NQ],
                                         lhsT=Wqp[:, c, 128 * m:128 * (m + 1)],
                                         rhs=efqT[:, c, :],
                                         start=(c == 0), stop=(c == 1))
                    nc.vector.tensor_scalar_add(QT[:, m, :], qps[:, 0:NQ], bqT[:, m, :])
                # V natural [kl, hid] in chunks of 120
                for ck in range(NCH):
                    P = _chunk_p(ck)
                    vps = vpp.tile([128, 512], FP, tag="vps")
                    for c in range(2):
                        nc.tensor.matmul(out=vps[0:P, 0:HID],
                                         lhsT=efT[:, c, CHUNK * ck:CHUNK * ck + P],
                                         rhs=Wvp[:, c, :], start=(c == 0), stop=False)
                    nc.tensor.matmul(out=vps[0:P, 0:HID], lhsT=onescol_bf[:, 0:P],
                                     rhs=bvrow[:], start=False, stop=True)
                    nc.vector.tensor_copy(Vt[0:P, ck, :], vps[0:P, 0:HID])

            if stage < 3:
                pass
            # =============== attention (per quad of 4 heads) ===============
            with tc.tile_pool(name="stpsum", bufs=1, space="PSUM") as stp_pool, \
                 tc.tile_pool(name="sumpsum", bufs=2, space="PSUM") as sum_pool, \
                 tc.tile_pool(name="upsum", bufs=2, space="PSUM") as u_pool:
                for q in range(2):
                    ETq = ETs[q]
                    sums_ps = [sum_pool.tile([128, NQ], FP, tag="sums",
                                             name=f"sums_{q}_{p_}") for p_ in range(2)]
                    for pair in range(2):
                        nc.tensor.matmul(out=sums_ps[pair][:], lhsT=zrow[:],
                                         rhs=onesq[:], start=True, stop=False,
                                         skip_group_check=True)
                    for ck in range(NCH):
                        P = _chunk_p(ck)
                        stp = stp_pool.tile([128, 4, 512], FP, tag="stp")
                        for hh in range(4):
                            nc.tensor.matmul(
                                out=stp[0:P, hh, 0:NQ],
                                lhsT=KT[32 * hh:32 * hh + 32, q, CHUNK * ck:CHUNK * ck + P],
                                rhs=QT[32 * hh:32 * hh + 32, q, :],
                                tile_position=(32 * hh, 0), start=True, stop=True)
                        # E = exp(alpha * S), straight from PSUM, strided over 4 heads
                        nc.scalar.activation(out=ETq[0:P, :, ck, :], in_=stp[0:P, :, 0:NQ],
                                             func=AF.Exp, scale=float(ALPHA))
                        # denominators: sums[k, q] via segment-indicator matmul
                        s0 = 39 - 3 * ck
                        for pair in range(2):
                            for t in range(2):
                                hh = 2 * pair + t
                                nc.tensor.matmul(
                                    out=sums_ps[pair][64 * t:64 * t + 64, :],
                                    lhsT=iseg[0:P, s0:s0 + 64],
                                    rhs=ETq[0:P, hh, ck, :],
                                    tile_position=(0, 64 * t),
                                    start=False, stop=False,
                                    skip_group_check=True)
                    for pair in range(2):
                        nc.tensor.matmul(out=sums_ps[pair][:], lhsT=zrow[:],
                                         rhs=onesq[:], start=False, stop=True,
                                         skip_group_check=True)
                    # reciprocal -> bf16, bounce through DRAM
                    for pair in range(2):
                        with nc.allow_low_precision(reason="softmax recip to bf16"):
                            nc.vector.reciprocal(rq[:, q, pair, :], sums_ps[pair][:])
                    nc.sync.dma_start(out=r_dram[q], in_=rq[:, q, :, :])
                    # expand over l: R[p, h, ck, qq] = r[3ck + p//40 (+64*(h%2)), h//2, qq]
                    for hh in range(4):
                        pair, t = hh // 2, hh % 2
                        base = q * (128 * 2 * NQ) + t * (64 * 2 * NQ) + pair * NQ
                        for kb in range(3):
                            src = bass.AP(
                                tensor=r_dram,
                                offset=base + kb * (2 * NQ),
                                ap=[[0, 40], [2 * NQ * 3, NCH], [1, NQ]],
                            )
                            nc.sync.dma_start(out=PT[40 * kb:40 * kb + 40, hh, :, :].opt(),
                                              in_=src)
                    # P^T = E^T * R   (bf16, unit-stride, 2x mode)
                    for hh in range(4):
                        nc.vector.tensor_mul(PT[:, hh, 0:NCH - 1, :],
                                             ETq[:, hh, 0:NCH - 1, :],
                                             PT[:, hh, 0:NCH - 1, :])
                        nc.vector.tensor_mul(PT[0:LAST_P, hh, NCH - 1, :],
                                             ETq[0:LAST_P, hh, NCH - 1, :],
                                             PT[0:LAST_P, hh, NCH - 1, :])
                    # U^T: contract kl, 4 heads col-tiled -> attT[:, q, :]
                    ups = u_pool.tile([128, NQ], FP, tag="ups")
                    nc.tensor.matmul(out=ups[:], lhsT=zrow[:], rhs=onesq[:],
                                     start=True, stop=False, skip_group_check=True)
                    for ck in range(NCH):
                        P = _chunk_p(ck)
                        for hh in range(4):
                            nc.tensor.matmul(
                                out=ups[32 * hh:32 * hh + 32, :],
                                lhsT=Vt[0:P, ck, 128 * q + 32 * hh:128 * q + 32 * hh + 32],
                                rhs=PT[0:P, hh, ck, :],
                                tile_position=(0, 32 * hh),
                                start=False, stop=False,
                                skip_group_check=True)
                    nc.tensor.matmul(out=ups[:], lhsT=zrow[:], rhs=onesq[:],
                                     start=False, stop=True, skip_group_check=True)
                    nc.vector.tensor_copy(attT[:, q, :], ups[:])

            if stage < 4:
                pass
            # =============== phase 3: gate + output projection ===============
            with tc.tile_pool(name="p3psum", bufs=2, space="PSUM") as p3p:
                for m in range(2):
                    gps = p3p.tile([128, NQ], FP, tag="gps")
                    for c in range(2):
                        nc.tensor.matmul(out=gps[:],
                                         lhsT=Wtgb[:, c, 128 * m:128 * (m + 1)],
                                         rhs=attT[:, c, :], start=(c == 0), stop=False)
                    nc.tensor.matmul(out=gps[:], lhsT=b_sb["btg"][:, 128 * m:128 * (m + 1)],
                                     rhs=onesq[:], start=False, stop=True)
                    nc.scalar.activation(out=gateT[:, m, :], in_=gps[:], func=AF.Exp,
                                         scale=-1.0)
                    nc.vector.tensor_scalar_add(gateT[:, m, :], gateT[:, m, :], 1.0)
                    with nc.allow_low_precision(reason="sigmoid recip bf16"):
                        nc.vector.reciprocal(gateT[:, m, :], gateT[:, m, :])
                    nc.vector.tensor_mul(gatedT[:, m, :], attT[:, m, :], gateT[:, m, :])
                # out[q, :] = gated @ Wo + bo
                with tc.tile_pool(name="outst", bufs=2) as ost:
                    for qc in range(4):
                        o0 = qc * 128
                        on = min(128, NQ - o0)
                        ops = p3p.tile([128, HID], FP, tag="ops")
                        for c in range(2):
                            nc.tensor.matmul(out=ops[0:on, :],
                                             lhsT=gatedT[:, c, o0:o0 + on],
                                             rhs=Wob[:, c, :], start=(c == 0), stop=False)
                        nc.tensor.matmul(out=ops[0:on, :], lhsT=ones128[:, 0:on],
                                         rhs=b_sb["bo"][:], start=False, stop=True)
                        osb = ost.tile([128, HID], FP, tag="osb")
                        nc.vector.tensor_copy(osb[0:on, :], ops[0:on, :])
                        nc.sync.dma_start(out=out[o0:o0 + on, :], in_=osb[0:on, :])

    nc.finalize()
    return nc


def _get_compiled():
    global _COMPILED
    if _COMPILED is None:
        _COMPILED = _build_nc()
    return _COMPILED


def _numpy_reference(edge_features, edge_mask, condition, Wq, bq, Wk, bk, Wv, bv,
                     Wcp, bcp, Wcg, bcg, Wtb, btb, Wtg, btg, Wo, bo):
    def sig(x):
        return 1.0 / (1.0 + np.exp(-x))
    cond_proj = condition @ Wcp + bcp
    cond_gate = sig(condition @ Wcg + bcg)
    cf = edge_features * cond_gate[:, None, None, :] + cond_proj[:, None, None, :]
    Q = (cf @ Wq + bq).reshape(B, N, N, NH, HD)
    K = (cf @ Wk + bk).reshape(B, N, N, NH, HD)
    V = (cf @ Wv + bv).reshape(B, N, N, NH, HD)
    scores = np.einsum('bijhd,bklhd->bijklh', Q, K) / np.sqrt(HD).astype(np.float32)
    bias_in = np.concatenate(
        [cf, np.broadcast_to(condition[:, None, None, :], (B, N, N, CD))], axis=-1)
    bias = bias_in @ Wtb + btb
    scores = scores + bias[:, :, :, None, None, :]
    m = edge_mask[:, None, None, :, :, None] & edge_mask[:, :, :, None, None, None]
    scores = np.where(m, scores, -np.inf)
    mx = np.max(scores, axis=4, keepdims=True)
    mx = np.where(np.isfinite(mx), mx, 0.0)
    e = np.exp(scores - mx)
    attn = e / np.maximum(np.sum(e, axis=4, keepdims=True), 1e-30)
    attended = np.einsum('bijklh,bklhd->bijhd', attn, V).reshape(B, N, N, HID)
    gate = sig(attended @ Wtg + btg)
    return ((attended * gate) @ Wo + bo).astype(np.float32)


def _make_in_maps(ins):
    ef_full = np.ascontiguousarray(ins["edge_features"].astype(np.float32)
                                   .reshape(B, KL, HID))
    condition = ins["condition"].astype(np.float32)

    def row(x):
        return np.ascontiguousarray(x.astype(np.float32).reshape(1, -1))

    shared = {
        "Wq": ins["Wq"].astype(np.float32), "Wk": ins["Wk"].astype(np.float32),
        "Wv": ins["Wv"].astype(np.float32), "Wtg": ins["Wtg"].astype(np.float32),
        "Wo": ins["Wo"].astype(np.float32), "Wcp": ins["Wcp"].astype(np.float32),
        "Wcg": ins["Wcg"].astype(np.float32),
        "bq": row(ins["bq"]), "bk": row(ins["bk"]), "bv": row(ins["bv"]),
        "btg": row(ins["btg"]), "bo": row(ins["bo"]),
        "bcp": row(ins["bcp"]), "bcg": row(ins["bcg"]),
    }
    in_maps = []
    for c in range(NCORES):
        b, s = c // 4, c % 4
        m = dict(shared)
        m["ef"] = np.ascontiguousarray(ef_full[b])
        m["efq"] = np.ascontiguousarray(ef_full[b, s * NQ:(s + 1) * NQ])
        m["cond"] = np.ascontiguousarray(condition[b:b + 1])
        in_maps.append(m)
    return in_maps


def kernel(**inputs):
    ins = {k: np.asarray(v) for k, v in inputs.items()}
    edge_mask = ins["edge_mask"]
    if not bool(edge_mask.all()):
        return _numpy_reference(
            ins["edge_features"].astype(np.float32), edge_mask.astype(bool),
            ins["condition"].astype(np.float32),
            *[ins[k].astype(np.float32) for k in
              ("Wq", "bq", "Wk", "bk", "Wv", "bv", "Wcp", "bcp", "Wcg", "bcg",
               "Wtb", "btb", "Wtg", "btg", "Wo", "bo")])

    in_maps = _make_in_maps(ins)
    from concourse.bass_utils import run_bass_kernel_spmd
    nc = _get_compiled()
    res = run_bass_kernel_spmd(nc, in_maps, core_ids=list(range(NCORES)))
    outs = [r["out"] for r in res.results]
    full = np.empty((B, KL, HID), np.float32)
    for c in range(NCORES):
        b, s = c // 4, c % 4
        full[b, s * NQ:(s + 1) * NQ] = outs[c]
    return full.reshape(B, N, N, HID)


if __name__ == "__main__":
    nc = _build_nc()
    print("built ok")



# revision 3
# speedup vs baseline: 1.7818x; 1.7818x over previous
"""Trainium2 Bass kernel for ConditionalTriangleAttention.

Reference computation (B=2, N=40, HID=256, NH=8, CD=128, HD=32):
  cf = edge_features * sigmoid(cond@Wcg+bcg) + (cond@Wcp+bcp)     (per batch)
  Q/K/V = cf @ W_{q,k,v} + b                                       [B,N,N,NH,HD]
  scores = einsum('bijhd,bklhd->bijklh', Q, K)/sqrt(HD) + bias     (bias const over k,l)
  attn = softmax over l;  attended = einsum('bijklh,bklhd->bijhd', attn, V)
  out = (attended * sigmoid(attended@Wtg+btg)) @ Wo + bo

With edge_mask all-ones (guaranteed by the input spec) the additive bias is
constant along the softmax axis and cancels, so Wtb/btb/edge_mask are no-ops.
A numpy fallback handles any other mask.

Sharding: 8 cores, each owns 400 query rows (b = core//4, i-rows slice) and
computes all heads for them end-to-end -- no collectives.

Per-core pipeline (v2): 4 units of (head-quad, head-pair), software-pipelined
A0 KQV A1 B0 A2 B1 A3 B2 B3 where
  A(u): per kl-chunk S^T matmul (PE, double-buffered PSUM) -> exp (ACT) ->
        per-k sums via indicator matmul (PE, k indexed as 14*kb+ck so the
        reciprocal lands DRAM-contiguous); reciprocal (DVE); r bounce to DRAM
        (SP queue); l-broadcast expansion back to SBUF as TWO large-descriptor
        DMAs on the gpsimd queue (11.2KB descriptors).
  B(u): P^T = E^T * R (DVE bf16 2x) ; U^T += V^T-contraction (PE).
"""

import os
import sys

for _p in ("/opt/trn_rl_repo", "/root/.axon_site/_ro/trn_rl_repo"):
    if os.path.isdir(_p) and _p not in sys.path:
        sys.path.insert(0, _p)

import numpy as np

B, N, HID, NH, CD = 2, 40, 256, 8, 128
HD = HID // NH            # 32
KL = N * N                # 1600
NQ = KL // 4              # 400 query rows per core
NCORES = 8
ALPHA = 1.0 / np.sqrt(np.float32(HD))

CHUNK = 120               # kl chunk: 3 k-groups of 40
NCH = 14                  # 13*120 + 40
LAST_P = KL - (NCH - 1) * CHUNK   # 40
CKQ = NCH * NQ            # 5600

_COMPILED = None


def _chunk_p(ck):
    return CHUNK if ck < NCH - 1 else LAST_P


def _make_ind():
    # ind[p, c] = 1 iff c == 14*(p//40) + 13.  Window ind[:, 13-ck : 77-ck]
    # as matmul lhsT maps chunk partition p -> sums row jj = 14*(p//40) + ck.
    ind = np.zeros((CHUNK, 77), np.float32)
    for p in range(CHUNK):
        ind[p, 14 * (p // 40) + 13] = 1.0
    return ind


def _build_nc():
    import concourse.bass as bass
    import concourse.tile as tile
    from concourse import bacc, mybir
    from concourse.masks import make_identity

    FP = mybir.dt.float32
    BF = mybir.dt.bfloat16
    AF = mybir.ActivationFunctionType

    nc = bacc.Bacc(None, target_bir_lowering=False)

    ef = nc.dram_tensor("ef", [KL, HID], FP, kind="ExternalInput")
    efq = nc.dram_tensor("efq", [NQ, HID], FP, kind="ExternalInput")
    cond = nc.dram_tensor("cond", [1, CD], FP, kind="ExternalInput")
    ind = nc.dram_tensor("ind", [CHUNK, 77], FP, kind="ExternalInput")
    Wq = nc.dram_tensor("Wq", [HID, HID], FP, kind="ExternalInput")
    Wk = nc.dram_tensor("Wk", [HID, HID], FP, kind="ExternalInput")
    Wv = nc.dram_tensor("Wv", [HID, HID], FP, kind="ExternalInput")
    Wtg = nc.dram_tensor("Wtg", [HID, HID], FP, kind="ExternalInput")
    Wo = nc.dram_tensor("Wo", [HID, HID], FP, kind="ExternalInput")
    Wcp = nc.dram_tensor("Wcp", [CD, HID], FP, kind="ExternalInput")
    Wcg = nc.dram_tensor("Wcg", [CD, HID], FP, kind="ExternalInput")
    bq = nc.dram_tensor("bq", [1, HID], FP, kind="ExternalInput")
    bk = nc.dram_tensor("bk", [1, HID], FP, kind="ExternalInput")
    bv = nc.dram_tensor("bv", [1, HID], FP, kind="ExternalInput")
    btg = nc.dram_tensor("btg", [1, HID], FP, kind="ExternalInput")
    bo = nc.dram_tensor("bo", [1, HID], FP, kind="ExternalInput")
    bcp = nc.dram_tensor("bcp", [1, HID], FP, kind="ExternalInput")
    bcg = nc.dram_tensor("bcg", [1, HID], FP, kind="ExternalInput")
    out = nc.dram_tensor("out", [NQ, HID], FP, kind="ExternalOutput")

    # r bounce: per (quad, pair, t): [3 kb][NCH*NQ] contiguous runs
    r2 = nc.dram_tensor("r2", [8, 3 * CKQ], BF, kind="Internal")

    with tile.TileContext(nc) as tc:
        with tc.tile_pool(name="persist", bufs=1) as sb:
            ident = sb.tile([128, 128], FP, tag="ident")
            make_identity(nc, ident)
            ones11 = sb.tile([1, 1], FP, tag="ones11")
            nc.vector.memset(ones11, 1.0)
            onesq = sb.tile([1, NQ], FP, tag="onesq")
            nc.vector.memset(onesq, 1.0)
            ones128 = sb.tile([1, 128], FP, tag="ones128")
            nc.vector.memset(ones128, 1.0)
            onescol_bf = sb.tile([1, CHUNK], BF, tag="onescol")
            nc.vector.memset(onescol_bf, 1.0)
            zrow = sb.tile([1, 128], FP, tag="zrow")
            nc.vector.memset(zrow, 0.0)

            ind_sb = sb.tile([CHUNK, 77], BF, tag="ind_sb")

            # persistent bf16 operands
            efT = sb.tile([128, 2, KL], BF, tag="efT")
            efqT = sb.tile([128, 2, NQ], BF, tag="efqT")
            KT = sb.tile([128, 2, KL], BF, tag="KT")
            QT = sb.tile([128, 2, NQ], BF, tag="QT")
            Vt = sb.tile([128, NCH, HID], BF, tag="Vt")
            attT = sb.tile([128, 2, NQ], BF, tag="attT")
            gateT = sb.tile([128, 2, NQ], BF, tag="gateT")
            gatedT = sb.tile([128, 2, NQ], BF, tag="gatedT")
            # gated projection weights (bf16)
            Wqp = sb.tile([128, 2, HID], BF, tag="Wqp")
            Wkp = sb.tile([128, 2, HID], BF, tag="Wkp")
            Wvp = sb.tile([128, 2, HID], BF, tag="Wvp")
            Wtgb = sb.tile([128, 2, HID], BF, tag="Wtgb")
            Wob = sb.tile([128, 2, HID], BF, tag="Wob")
            # per-partition columns
            gT = sb.tile([128, 2, 1], FP, tag="gT")
            pT = sb.tile([128, 2, 1], FP, tag="pT")
            bqT = sb.tile([128, 2, 1], FP, tag="bqT")
            bkT = sb.tile([128, 2, 1], FP, tag="bkT")
            bvrow = sb.tile([1, HID], BF, tag="bvrow")
            btg_sb = sb.tile([1, HID], FP, tag="b_btg")
            bo_sb = sb.tile([1, HID], FP, tag="b_bo")

            nc.scalar.dma_start(out=btg_sb[:], in_=btg[:])
            nc.scalar.dma_start(out=bo_sb[:], in_=bo[:])

            # =============== phase 1: staging + cond + transposes ===============
            with tc.tile_pool(name="wstage", bufs=1) as ws, \
                 tc.tile_pool(name="p1psum", bufs=2, space="PSUM") as pp, \
                 tc.tile_pool(name="p1psumB", bufs=2, space="PSUM") as ppB, \
                 tc.tile_pool(name="efstage", bufs=4) as efst:

                ind_f = ws.tile([CHUNK, 77], FP, tag="ind_f")
                nc.scalar.dma_start(out=ind_f[:], in_=ind[:])
                nc.vector.tensor_copy(ind_sb[:], ind_f[:])

                w_sb = {}
                for name, t, pdim in (("Wq", Wq, HID), ("Wk", Wk, HID),
                                      ("Wv", Wv, HID), ("Wtg", Wtg, HID),
                                      ("Wo", Wo, HID), ("Wcp", Wcp, CD),
                                      ("Wcg", Wcg, CD)):
                    nch = pdim // 128
                    tl = ws.tile([128, nch, HID], FP, tag="w_" + name)
                    for m in range(nch):
                        nc.scalar.dma_start(out=tl[:, m, :],
                                            in_=t[128 * m:128 * (m + 1), :])
                    w_sb[name] = tl
                b_sb = {}
                for name, t in (("bq", bq), ("bk", bk), ("bv", bv),
                                ("bcp", bcp), ("bcg", bcg)):
                    tl = ws.tile([1, HID], FP, tag="b_" + name)
                    nc.scalar.dma_start(out=tl[:], in_=t[:])
                    b_sb[name] = tl
                cond_sb = ws.tile([1, CD], FP, tag="cond_sb")
                nc.scalar.dma_start(out=cond_sb[:], in_=cond[:])

                # condT column [128,1]
                ct_ps = pp.tile([128, 1], FP, tag="tiny")
                nc.tensor.matmul(out=ct_ps[:], lhsT=cond_sb[:], rhs=ones11[:],
                                 start=True, stop=True)
                condT = ws.tile([128, 1], FP, tag="condT")
                nc.vector.tensor_copy(condT[:], ct_ps[:])

                # gates / proj columns per hid chunk
                for m in range(2):
                    gp_ps = pp.tile([128, 1], FP, tag="tiny")
                    nc.tensor.matmul(out=gp_ps[:],
                                     lhsT=w_sb["Wcg"][:, 0, 128 * m:128 * (m + 1)],
                                     rhs=condT[:], start=True, stop=False)
                    nc.tensor.matmul(out=gp_ps[:],
                                     lhsT=b_sb["bcg"][:, 128 * m:128 * (m + 1)],
                                     rhs=ones11[:], start=False, stop=True)
                    nc.scalar.activation(out=gT[:, m, :], in_=gp_ps[:],
                                         func=AF.Sigmoid)

                    pp_ps = pp.tile([128, 1], FP, tag="tiny")
                    nc.tensor.matmul(out=pp_ps[:],
                                     lhsT=w_sb["Wcp"][:, 0, 128 * m:128 * (m + 1)],
                                     rhs=condT[:], start=True, stop=False)
                    nc.tensor.matmul(out=pp_ps[:],
                                     lhsT=b_sb["bcp"][:, 128 * m:128 * (m + 1)],
                                     rhs=ones11[:], start=False, stop=True)
                    nc.vector.tensor_copy(pT[:, m, :], pp_ps[:])

                # gated weights W' = diag(g) W  (bf16), per input-chunk m
                for (wn, dst) in (("Wq", Wqp), ("Wk", Wkp), ("Wv", Wvp)):
                    for m in range(2):
                        nc.vector.tensor_scalar_mul(dst[:, m, :], w_sb[wn][:, m, :],
                                                    gT[:, m, :])
                for (wn, dst) in (("Wtg", Wtgb), ("Wo", Wob)):
                    for m in range(2):
                        nc.vector.tensor_copy(dst[:, m, :], w_sb[wn][:, m, :])

                # bias columns b' = (p @ W + b)^T  for q,k
                for (wn, bn, dst) in (("Wq", "bq", bqT), ("Wk", "bk", bkT)):
                    for m in range(2):
                        bps = pp.tile([128, 1], FP, tag="tiny")
                        for c in range(2):
                            nc.tensor.matmul(out=bps[:],
                                             lhsT=w_sb[wn][:, c, 128 * m:128 * (m + 1)],
                                             rhs=pT[:, c, :], start=(c == 0),
                                             stop=False)
                        nc.tensor.matmul(out=bps[:],
                                         lhsT=b_sb[bn][:, 128 * m:128 * (m + 1)],
                                         rhs=ones11[:], start=False, stop=True)
                        nc.vector.tensor_copy(dst[:, m, :], bps[:])
                # bv' as a row (used via rank-1 matmul into V)
                bvr_ps = pp.tile([1, HID], FP, tag="tiny")
                for c in range(2):
                    nc.tensor.matmul(out=bvr_ps[:], lhsT=pT[:, c, :],
                                     rhs=w_sb["Wv"][:, c, :], start=(c == 0),
                                     stop=False)
                nc.tensor.matmul(out=bvr_ps[:], lhsT=ones11[:], rhs=b_sb["bv"][:],
                                 start=False, stop=True)
                nc.vector.tensor_copy(bvrow[:], bvr_ps[:])

                # ---- transpose ef and efq into efT / efqT (bf16) ----
                def do_transpose(src_dram, nrows, dstT):
                    ntile = (nrows + 127) // 128
                    for qt in range(ntile):
                        r0 = qt * 128
                        rn = min(128, nrows - r0)
                        stg = efst.tile([128, HID], FP, tag="efstg")
                        nc.sync.dma_start(out=stg[0:rn, :],
                                          in_=src_dram[r0:r0 + rn, :])
                        for m in range(2):
                            tp = ppB.tile([128, 128], FP, tag="tp")
                            nc.tensor.transpose(out=tp[:, 0:rn],
                                                in_=stg[0:rn, 128 * m:128 * (m + 1)],
                                                identity=ident[0:rn, 0:rn])
                            nc.vector.tensor_copy(dstT[:, m, r0:r0 + rn],
                                                  tp[:, 0:rn])

                do_transpose(ef, KL, efT)
                do_transpose(efq, NQ, efqT)

            # =============== phase 2: K^T and Q^T projections ===============
            with tc.tile_pool(name="kpsum", bufs=1, space="PSUM") as kpp, \
                 tc.tile_pool(name="projpsum", bufs=2, space="PSUM") as prp:
                # K^T (all heads) [hid', kl], bias via ACT copy-with-bias
                for m in range(2):
                    kps = kpp.tile([128, 2048], FP, tag="kps")
                    for (o, w) in ((0, 512), (512, 512), (1024, 512), (1536, 64)):
                        for c in range(2):
                            nc.tensor.matmul(out=kps[:, o:o + w],
                                             lhsT=Wkp[:, c, 128 * m:128 * (m + 1)],
                                             rhs=efT[:, c, o:o + w],
                                             start=(c == 0), stop=(c == 1))
                    nc.scalar.activation(out=KT[:, m, :], in_=kps[:, 0:KL],
                                         func=AF.Identity, bias=bkT[:, m, :])
                # Q^T from efqT
                for m in range(2):
                    qps = prp.tile([128, 512], FP, tag="qps")
                    for c in range(2):
                        nc.tensor.matmul(out=qps[:, 0:NQ],
                                         lhsT=Wqp[:, c, 128 * m:128 * (m + 1)],
                                         rhs=efqT[:, c, :],
                                         start=(c == 0), stop=(c == 1))
                    nc.scalar.activation(out=QT[:, m, :], in_=qps[:, 0:NQ],
                                         func=AF.Identity, bias=bqT[:, m, :])

            # =============== attention: 4 software-pipelined units ===============
            with tc.tile_pool(name="stp", bufs=2, space="PSUM") as stp_pool, \
                 tc.tile_pool(name="sums", bufs=2, space="PSUM") as sum_pool, \
                 tc.tile_pool(name="misc", bufs=2, space="PSUM") as misc_pool, \
                 tc.tile_pool(name="Epool", bufs=3) as E_pool, \
                 tc.tile_pool(name="Ppool", bufs=3) as P_pool, \
                 tc.tile_pool(name="rqpool", bufs=2) as rq_pool:

                units = [(qd, pair) for qd in range(2) for pair in range(2)]
                Etiles = {}
                Ptiles = {}
                upst = {}

                def emit_V():
                    # V natural [kl, hid] in chunks of 120 (misc psum pool)
                    for ck in range(NCH):
                        P = _chunk_p(ck)
                        vps = misc_pool.tile([128, 512], FP, tag="misc",
                                             name=f"vps{ck}")
                        for c in range(2):
                            nc.tensor.matmul(out=vps[0:P, 0:HID],
                                             lhsT=efT[:, c, CHUNK * ck:CHUNK * ck + P],
                                             rhs=Wvp[:, c, :], start=(c == 0),
                                             stop=False)
                        nc.tensor.matmul(out=vps[0:P, 0:HID],
                                         lhsT=onescol_bf[:, 0:P], rhs=bvrow[:],
                                         start=False, stop=True)
                        nc.vector.tensor_copy(Vt[0:P, ck, :], vps[0:P, 0:HID])

                def emit_A(u):
                    qd, pair = units[u]
                    E = E_pool.tile([CHUNK, 2, NCH, NQ], BF, tag="E",
                                    name=f"E{u}")
                    Etiles[u] = E
                    sums = sum_pool.tile([128, NQ], FP, tag="sums",
                                         name=f"sums{u}")
                    nc.tensor.matmul(out=sums[:], lhsT=zrow[:], rhs=onesq[:],
                                     start=True, stop=False,
                                     skip_group_check=True)
                    for ck in range(NCH):
                        P = _chunk_p(ck)
                        stp = stp_pool.tile([128, 2, 512], FP, tag="stp",
                                            name=f"stp{u}_{ck}")
                        for t in range(2):
                            hh = 2 * pair + t
                            nc.tensor.matmul(
                                out=stp[0:P, t, 0:NQ],
                                lhsT=KT[32 * hh:32 * hh + 32, qd,
                                        CHUNK * ck:CHUNK * ck + P],
                                rhs=QT[32 * hh:32 * hh + 32, qd, :],
                                tile_position=(32 * hh, 0),
                                start=True, stop=True)
                        # E = exp(alpha * S) for both heads of the pair
                        nc.scalar.activation(out=E[0:P, :, ck, :],
                                             in_=stp[0:P, :, 0:NQ],
                                             func=AF.Exp, scale=float(ALPHA))
                        # per-k sums, k indexed as jj = 14*kb + ck
                        for t in range(2):
                            nc.tensor.matmul(
                                out=sums[64 * t:64 * t + 64, :],
                                lhsT=ind_sb[0:P, 13 - ck:77 - ck],
                                rhs=E[0:P, t, ck, :],
                                tile_position=(0, 64 * t),
                                start=False, stop=False,
                                skip_group_check=True)
                    nc.tensor.matmul(out=sums[:], lhsT=zrow[:], rhs=onesq[:],
                                     start=False, stop=True,
                                     skip_group_check=True)
                    # reciprocal -> bf16
                    rq = rq_pool.tile([128, NQ], BF, tag="rq", name=f"rq{u}")
                    with nc.allow_low_precision(reason="softmax recip to bf16"):
                        nc.vector.reciprocal(rq[:], sums[:])
                    # bounce r rows (t, jj=14kb+ck) to DRAM, contiguous per t
                    for t in range(2):
                        dst = bass.AP(tensor=r2,
                                      offset=(u * 2 + t) * 3 * CKQ,
                                      ap=[[400, 42], [1, 400]])
                        nc.sync.dma_start(out=dst, in_=rq[64 * t:64 * t + 42, :])
                    # l-broadcast expansion: DRAM -> P tile, big descriptors
                    Pt = P_pool.tile([CHUNK, 2, NCH, NQ], BF, tag="P",
                                     name=f"P{u}")
                    Ptiles[u] = Pt
                    for t in range(2):
                        src = bass.AP(tensor=r2,
                                      offset=(u * 2 + t) * 3 * CKQ,
                                      ap=[[CKQ, 3], [0, 40], [1, CKQ]])
                        nc.gpsimd.dma_start(out=Pt[:, t, :, :].opt(), in_=src)

                def emit_B(u):
                    qd, pair = units[u]
                    E = Etiles.pop(u)
                    Pt = Ptiles.pop(u)
                    # P = E * R  (bf16 2x), split so U can start early
                    nc.vector.tensor_mul(Pt[:, :, 0:7, :], E[:, :, 0:7, :],
                                         Pt[:, :, 0:7, :])
                    nc.vector.tensor_mul(Pt[:, :, 7:13, :], E[:, :, 7:13, :],
                                         Pt[:, :, 7:13, :])
                    nc.vector.tensor_mul(Pt[0:LAST_P, :, 13, :],
                                         E[0:LAST_P, :, 13, :],
                                         Pt[0:LAST_P, :, 13, :])
                    # U^T accumulation (one PSUM group per quad)
                    if pair == 0:
                        ups = misc_pool.tile([128, NQ], FP, tag="misc",
                                             name=f"ups{qd}")
                        upst[qd] = ups
                        nc.tensor.matmul(out=ups[:], lhsT=zrow[:], rhs=onesq[:],
                                         start=True, stop=False,
                                         skip_group_check=True)
                    ups = upst[qd]
                    for ck in range(NCH):
                        P = _chunk_p(ck)
                        for t in range(2):
                            hh = 2 * pair + t
                            nc.tensor.matmul(
                                out=ups[32 * hh:32 * hh + 32, :],
                                lhsT=Vt[0:P, ck,
                                        128 * qd + 32 * hh:128 * qd + 32 * hh + 32],
                                rhs=Pt[0:P, t, ck, :],
                                tile_position=(0, 32 * hh),
                                start=False, stop=False,
                                skip_group_check=True)
                    if pair == 1:
                        nc.tensor.matmul(out=ups[:], lhsT=zrow[:], rhs=onesq[:],
                                         start=False, stop=True,
                                         skip_group_check=True)
                        nc.vector.tensor_copy(attT[:, qd, :], ups[:])
                        del upst[qd]

                emit_A(0)
                emit_V()
                emit_A(1)
                emit_B(0)
                emit_A(2)
                emit_B(1)
                emit_A(3)
                emit_B(2)
                emit_B(3)

            # =============== phase 3: gate + output projection ===============
            with tc.tile_pool(name="p3psum", bufs=2, space="PSUM") as p3p, \
                 tc.tile_pool(name="outst", bufs=2) as ost:
                for m in range(2):
                    gps = p3p.tile([128, NQ], FP, tag="gps")
                    for c in range(2):
                        nc.tensor.matmul(out=gps[:],
                                         lhsT=Wtgb[:, c, 128 * m:128 * (m + 1)],
                                         rhs=attT[:, c, :], start=(c == 0),
                                         stop=False)
                    nc.tensor.matmul(out=gps[:],
                                     lhsT=btg_sb[:, 128 * m:128 * (m + 1)],
                                     rhs=onesq[:], start=False, stop=True)
                    nc.scalar.activation(out=gateT[:, m, :], in_=gps[:],
                                         func=AF.Sigmoid)
                    nc.vector.tensor_mul(gatedT[:, m, :], attT[:, m, :],
                                         gateT[:, m, :])
                # out[q, :] = gated @ Wo + bo
                for qc in range(4):
                    o0 = qc * 128
                    on = min(128, NQ - o0)
                    ops = p3p.tile([128, HID], FP, tag="ops")
                    for c in range(2):
                        nc.tensor.matmul(out=ops[0:on, :],
                                         lhsT=gatedT[:, c, o0:o0 + on],
                                         rhs=Wob[:, c, :], start=(c == 0),
                                         stop=False)
                    nc.tensor.matmul(out=ops[0:on, :], lhsT=ones128[:, 0:on],
                                     rhs=bo_sb[:], start=False, stop=True)
                    osb = ost.tile([128, HID], FP, tag="osb")
                    nc.vector.tensor_copy(osb[0:on, :], ops[0:on, :])
                    nc.sync.dma_start(out=out[o0:o0 + on, :], in_=osb[0:on, :])

    nc.finalize()
    return nc


def _get_compiled():
    global _COMPILED
    if _COMPILED is None:
        _COMPILED = _build_nc()
    return _COMPILED


def _numpy_reference(edge_features, edge_mask, condition, Wq, bq, Wk, bk, Wv, bv,
                     Wcp, bcp, Wcg, bcg, Wtb, btb, Wtg, btg, Wo, bo):
    def sig(x):
        return 1.0 / (1.0 + np.exp(-x))
    cond_proj = condition @ Wcp + bcp
    cond_gate = sig(condition @ Wcg + bcg)
    cf = edge_features * cond_gate[:, None, None, :] + cond_proj[:, None, None, :]
    Q = (cf @ Wq + bq).reshape(B, N, N, NH, HD)
    K = (cf @ Wk + bk).reshape(B, N, N, NH, HD)
    V = (cf @ Wv + bv).reshape(B, N, N, NH, HD)
    scores = np.einsum('bijhd,bklhd->bijklh', Q, K) / np.sqrt(HD).astype(np.float32)
    bias_in = np.concatenate(
        [cf, np.broadcast_to(condition[:, None, None, :], (B, N, N, CD))], axis=-1)
    bias = bias_in @ Wtb + btb
    scores = scores + bias[:, :, :, None, None, :]
    m = edge_mask[:, None, None, :, :, None] & edge_mask[:, :, :, None, None, None]
    scores = np.where(m, scores, -np.inf)
    mx = np.max(scores, axis=4, keepdims=True)
    mx = np.where(np.isfinite(mx), mx, 0.0)
    e = np.exp(scores - mx)
    attn = e / np.maximum(np.sum(e, axis=4, keepdims=True), 1e-30)
    attended = np.einsum('bijklh,bklhd->bijhd', attn, V).reshape(B, N, N, HID)
    gate = sig(attended @ Wtg + btg)
    return ((attended * gate) @ Wo + bo).astype(np.float32)


def _make_in_maps(ins):
    ef_full = np.ascontiguousarray(ins["edge_features"].astype(np.float32)
                                   .reshape(B, KL, HID))
    condition = ins["condition"].astype(np.float32)

    def row(x):
        return np.ascontiguousarray(x.astype(np.float32).reshape(1, -1))

    shared = {
        "Wq": ins["Wq"].astype(np.float32), "Wk": ins["Wk"].astype(np.float32),
        "Wv": ins["Wv"].astype(np.float32), "Wtg": ins["Wtg"].astype(np.float32),
        "Wo": ins["Wo"].astype(np.float32), "Wcp": ins["Wcp"].astype(np.float32),
        "Wcg": ins["Wcg"].astype(np.float32),
        "bq": row(ins["bq"]), "bk": row(ins["bk"]), "bv": row(ins["bv"]),
        "btg": row(ins["btg"]), "bo": row(ins["bo"]),
        "bcp": row(ins["bcp"]), "bcg": row(ins["bcg"]),
        "ind": _make_ind(),
    }
    in_maps = []
    for c in range(NCORES):
        b, s = c // 4, c % 4
        m = dict(shared)
        m["ef"] = np.ascontiguousarray(ef_full[b])
        m["efq"] = np.ascontiguousarray(ef_full[b, s * NQ:(s + 1) * NQ])
        m["cond"] = np.ascontiguousarray(condition[b:b + 1])
        in_maps.append(m)
    return in_maps


def kernel(**inputs):
    ins = {k: np.asarray(v) for k, v in inputs.items()}
    edge_mask = ins["edge_mask"]
    if not bool(edge_mask.all()):
        return _numpy_reference(
            ins["edge_features"].astype(np.float32), edge_mask.astype(bool),
            ins["condition"].astype(np.float32),
            *[ins[k].astype(np.float32) for k in
              ("Wq", "bq", "Wk", "bk", "Wv", "bv", "Wcp", "bcp", "Wcg", "bcg",
               "Wtb", "btb", "Wtg", "btg", "Wo", "bo")])

    in_maps = _make_in_maps(ins)
    from concourse.bass_utils import run_bass_kernel_spmd
    nc = _get_compiled()
    res = run_bass_kernel_spmd(nc, in_maps, core_ids=list(range(NCORES)))
    outs = [r["out"] for r in res.results]
    full = np.empty((B, KL, HID), np.float32)
    for c in range(NCORES):
        b, s = c // 4, c % 4
        full[b, s * NQ:(s + 1) * NQ] = outs[c]
    return full.reshape(B, N, N, HID)


if __name__ == "__main__":
    nc = _build_nc()
    print("built ok")


# revision 7
# speedup vs baseline: 1.8732x; 1.0513x over previous
"""Trainium2 Bass kernel for ConditionalTriangleAttention.

Reference computation (B=2, N=40, HID=256, NH=8, CD=128, HD=32):
  cf = edge_features * sigmoid(cond@Wcg+bcg) + (cond@Wcp+bcp)     (per batch)
  Q/K/V = cf @ W_{q,k,v} + b                                       [B,N,N,NH,HD]
  scores = einsum('bijhd,bklhd->bijklh', Q, K)/sqrt(HD) + bias     (bias const over k,l)
  attn = softmax over l;  attended = einsum('bijklh,bklhd->bijhd', attn, V)
  out = (attended * sigmoid(attended@Wtg+btg)) @ Wo + bo

With edge_mask all-ones (guaranteed by the input spec) the additive bias is
constant along the softmax axis and cancels, so Wtb/btb/edge_mask are no-ops.
A numpy fallback handles any other mask.

Sharding: 8 cores, each owns 400 query rows (b = core//4, i-rows slice) and
computes all heads for them end-to-end -- no collectives.

v3 pipeline per core: 4 units of (head-quad, head-pair), software-pipelined
A0 V A1 B0 A2 B1 A3 B2 B3.
  A(u): per kl-chunk S^T matmul (PE, double-buffered 2-bank PSUM) -> exp (ACT,
        the only ACT table used all kernel) -> per-k sums via indicator
        matmul (PE; k indexed 14*kb+ck so r lands DRAM-contiguous);
        reciprocal (DVE); r bounce to DRAM (SP); l-broadcast expansion as
        large-descriptor DMAs on the gpsimd queue.  The last unit splits its
        softmax tail into two ck-halves (second indicator, 7*kb+ck) to halve
        the end-of-pipe stall.
  B(u): P^T = E^T * R (DVE bf16 2x); U^T += V^T-contraction (PE).
Input DMAs are consolidated (one merged weight tensor, one merged bias row,
merged ef/efq loads); K^T is emitted interleaved with the ef transposes and
the attention pipeline starts after the m=0 half of K^T/Q^T.
"""

import os
import sys

for _p in ("/opt/trn_rl_repo", "/root/.axon_site/_ro/trn_rl_repo"):
    if os.path.isdir(_p) and _p not in sys.path:
        sys.path.insert(0, _p)

import numpy as np

B, N, HID, NH, CD = 2, 40, 256, 8, 128
HD = HID // NH            # 32
KL = N * N                # 1600
NQ = KL // 4              # 400 query rows per core
NCORES = 8
ALPHA = 1.0 / np.sqrt(np.float32(HD))

CHUNK = 120               # kl chunk: 3 k-groups of 40
NCH = 14                  # 13*120 + 40
LAST_P = KL - (NCH - 1) * CHUNK   # 40
CKQ = NCH * NQ            # 5600

_COMPILED = None


def _chunk_p(ck):
    return CHUNK if ck < NCH - 1 else LAST_P


def _make_ind():
    # indcat[:, 0, c]: 1 iff c == 14*(p//40) + 13   (full-unit sums layout)
    # indcat[:, 1, c]: 1 iff c ==  7*(p//40) + 13   (half-unit sums layout)
    ind = np.zeros((CHUNK, 2, 77), np.float32)
    for p in range(CHUNK):
        ind[p, 0, 14 * (p // 40) + 13] = 1.0
        ind[p, 1, 7 * (p // 40) + 13] = 1.0
    return ind


def _build_nc():
    import concourse.bass as bass
    import concourse.tile as tile
    from concourse import bacc, mybir
    from concourse.masks import make_identity

    FP = mybir.dt.float32
    BF = mybir.dt.bfloat16
    AF = mybir.ActivationFunctionType

    nc = bacc.Bacc(None, target_bir_lowering=False)

    ef = nc.dram_tensor("ef", [KL, HID], FP, kind="ExternalInput")
    efq = nc.dram_tensor("efq", [NQ, HID], FP, kind="ExternalInput")
    cond = nc.dram_tensor("cond", [1, CD], FP, kind="ExternalInput")
    ind = nc.dram_tensor("ind", [CHUNK, 2 * 77], FP, kind="ExternalInput")
    # wcat rows: Wq(256) Wk(256) Wv(256) Wtg(256) Wo(256) Wcp(128) Wcg(128)
    wcat = nc.dram_tensor("wcat", [12 * 128, HID], FP, kind="ExternalInput")
    # bcat rows: bq bk bv btg bo bcp bcg
    bcat = nc.dram_tensor("bcat", [1, 7 * HID], FP, kind="ExternalInput")
    out = nc.dram_tensor("out", [NQ, HID], FP, kind="ExternalOutput")

    # r bounce: slot u*2+t; full units use [3kb][CKQ]; the split unit (u=3)
    # uses [half][3kb][7*NQ].
    r2 = nc.dram_tensor("r2", [8, 3 * CKQ], BF, kind="Internal")

    W_OFF = {"Wq": 0, "Wk": 2, "Wv": 4, "Wtg": 6, "Wo": 8, "Wcp": 10, "Wcg": 11}
    B_OFF = {"bq": 0, "bk": 1, "bv": 2, "btg": 3, "bo": 4, "bcp": 5, "bcg": 6}

    with tile.TileContext(nc) as tc:
        with tc.tile_pool(name="persist", bufs=1) as sb:
            ident = sb.tile([128, 128], FP, tag="ident")
            make_identity(nc, ident)
            ones11 = sb.tile([1, 1], FP, tag="ones11")
            nc.vector.memset(ones11, 1.0)
            onesq = sb.tile([1, NQ], FP, tag="onesq")
            nc.vector.memset(onesq, 1.0)
            ones128 = sb.tile([1, 128], FP, tag="ones128")
            nc.vector.memset(ones128, 1.0)
            onescol_bf = sb.tile([1, CHUNK], BF, tag="onescol")
            nc.vector.memset(onescol_bf, 1.0)
            zrow = sb.tile([1, 128], FP, tag="zrow")
            nc.vector.memset(zrow, 0.0)

            ind_sb = sb.tile([CHUNK, 2, 77], BF, tag="ind_sb")

            # persistent bf16 operands
            efT = sb.tile([128, 2, KL], BF, tag="efT")
            efqT = sb.tile([128, 2, NQ], BF, tag="efqT")
            KT = sb.tile([128, 2, KL], BF, tag="KT")
            QT = sb.tile([128, 2, NQ], BF, tag="QT")
            Vt = sb.tile([128, NCH, HID], BF, tag="Vt")
            attT = sb.tile([128, 2, NQ], BF, tag="attT")
            gateT = sb.tile([128, 2, NQ], BF, tag="gateT")
            gatedT = sb.tile([128, 2, NQ], BF, tag="gatedT")
            # gated projection weights (bf16)
            Wqp = sb.tile([128, 2, HID], BF, tag="Wqp")
            Wkp = sb.tile([128, 2, HID], BF, tag="Wkp")
            Wvp = sb.tile([128, 2, HID], BF, tag="Wvp")
            Wtgb = sb.tile([128, 2, HID], BF, tag="Wtgb")
            Wob = sb.tile([128, 2, HID], BF, tag="Wob")
            # per-partition columns
            gT = sb.tile([128, 2, 1], FP, tag="gT")
            pT = sb.tile([128, 2, 1], FP, tag="pT")
            bqT = sb.tile([128, 2, 1], FP, tag="bqT")
            bkT = sb.tile([128, 2, 1], FP, tag="bkT")
            bvrow = sb.tile([1, HID], BF, tag="bvrow")
            bcat_sb = sb.tile([1, 7, HID], FP, tag="bcat_sb")
            osb = sb.tile([128, 4, HID], FP, tag="osb")

            # =============== phase 1: staging + cond + transposes + K/Q ========
            with tc.tile_pool(name="wstage", bufs=1) as ws, \
                 tc.tile_pool(name="p1psum", bufs=2, space="PSUM") as pp, \
                 tc.tile_pool(name="kqpsum", bufs=2, space="PSUM") as kqp, \
                 tc.tile_pool(name="tppsum", bufs=2, space="PSUM") as ppB:

                # ---- consolidated input loads ----
                nc.scalar.dma_start(out=bcat_sb[:, :, :].opt(),
                                    in_=bcat[:, :])
                ind_f = ws.tile([CHUNK, 2, 77], FP, tag="ind_f")
                nc.scalar.dma_start(out=ind_f[:, :, :].opt(),
                                    in_=ind[:, :])
                nc.vector.tensor_copy(ind_sb[:], ind_f[:])
                cond_sb = ws.tile([1, CD], FP, tag="cond_sb")
                nc.scalar.dma_start(out=cond_sb[:], in_=cond[:])

                wst = ws.tile([128, 12, HID], FP, tag="wst")
                nc.scalar.dma_start(
                    out=wst[:, :, :].opt(),
                    in_=bass.AP(tensor=wcat, offset=0,
                                ap=[[HID, 128], [128 * HID, 12], [1, HID]]))

                stg = ws.tile([128, 13, HID], FP, tag="stg")
                nc.sync.dma_start(
                    out=stg[:, 0:12, :].opt(),
                    in_=bass.AP(tensor=ef, offset=0,
                                ap=[[HID, 128], [128 * HID, 12], [1, HID]]))
                nc.sync.dma_start(out=stg[0:64, 12, :], in_=ef[1536:1600, :])
                stgq = ws.tile([128, 4, HID], FP, tag="stgq")
                nc.sync.dma_start(
                    out=stgq[:, 0:3, :].opt(),
                    in_=bass.AP(tensor=efq, offset=0,
                                ap=[[HID, 128], [128 * HID, 3], [1, HID]]))
                nc.sync.dma_start(out=stgq[0:16, 3, :], in_=efq[384:400, :])

                def wslice(name, m, c0=0, cn=HID):
                    return wst[:, W_OFF[name] + m, c0:c0 + cn]

                def bslice(name, c0=0, cn=HID):
                    return bcat_sb[:, B_OFF[name], c0:c0 + cn]

                # ---- conditional gating columns ----
                ct_ps = pp.tile([128, 1], FP, tag="tiny")
                nc.tensor.matmul(out=ct_ps[:], lhsT=cond_sb[:], rhs=ones11[:],
                                 start=True, stop=True)
                condT = ws.tile([128, 1], FP, tag="condT")
                nc.vector.tensor_copy(condT[:], ct_ps[:])

                for m in range(2):
                    gp_ps = pp.tile([128, 1], FP, tag="tiny")
                    nc.tensor.matmul(out=gp_ps[:],
                                     lhsT=wslice("Wcg", 0, 128 * m, 128),
                                     rhs=condT[:], start=True, stop=False)
                    nc.tensor.matmul(out=gp_ps[:],
                                     lhsT=bslice("bcg", 128 * m, 128),
                                     rhs=ones11[:], start=False, stop=True)
                    # sigmoid via exp so ACT keeps the Exp table all kernel
                    nc.scalar.activation(out=gT[:, m, :], in_=gp_ps[:],
                                         func=AF.Exp, scale=-1.0)
                    nc.vector.tensor_scalar_add(gT[:, m, :], gT[:, m, :], 1.0)
                    nc.vector.reciprocal(gT[:, m, :], gT[:, m, :])

                    pp_ps = pp.tile([128, 1], FP, tag="tiny")
                    nc.tensor.matmul(out=pp_ps[:],
                                     lhsT=wslice("Wcp", 0, 128 * m, 128),
                                     rhs=condT[:], start=True, stop=False)
                    nc.tensor.matmul(out=pp_ps[:],
                                     lhsT=bslice("bcp", 128 * m, 128),
                                     rhs=ones11[:], start=False, stop=True)
                    nc.vector.tensor_copy(pT[:, m, :], pp_ps[:])

                # gated weights W' = diag(g) W  (bf16)
                for (wn, dst) in (("Wq", Wqp), ("Wk", Wkp), ("Wv", Wvp)):
                    for m in range(2):
                        nc.vector.tensor_scalar_mul(dst[:, m, :],
                                                    wslice(wn, m), gT[:, m, :])
                for (wn, dst) in (("Wtg", Wtgb), ("Wo", Wob)):
                    for m in range(2):
                        nc.vector.tensor_copy(dst[:, m, :], wslice(wn, m))

                # bias columns b' = (p @ W + b)^T  for q,k
                for (wn, bn, dst) in (("Wq", "bq", bqT), ("Wk", "bk", bkT)):
                    for m in range(2):
                        bps = pp.tile([128, 1], FP, tag="tiny")
                        for c in range(2):
                            nc.tensor.matmul(out=bps[:],
                                             lhsT=wslice(wn, c, 128 * m, 128),
                                             rhs=pT[:, c, :], start=(c == 0),
                                             stop=False)
                        nc.tensor.matmul(out=bps[:],
                                         lhsT=bslice(bn, 128 * m, 128),
                                         rhs=ones11[:], start=False, stop=True)
                        nc.vector.tensor_copy(dst[:, m, :], bps[:])
                # bv' as a row (used via rank-1 matmul into V)
                bvr_ps = pp.tile([1, HID], FP, tag="tiny")
                for c in range(2):
                    nc.tensor.matmul(out=bvr_ps[:], lhsT=pT[:, c, :],
                                     rhs=wslice("Wv", c), start=(c == 0),
                                     stop=False)
                nc.tensor.matmul(out=bvr_ps[:], lhsT=ones11[:],
                                 rhs=bslice("bv"), start=False, stop=True)
                nc.vector.tensor_copy(bvrow[:], bvr_ps[:])

                # ---- transposes interleaved with K^T blocks ----
                def tp_tile(stg_tile, qt, rn, dstT):
                    for m in range(2):
                        tp = ppB.tile([128, 128], FP, tag="tp")
                        nc.tensor.transpose(out=tp[:, 0:rn],
                                            in_=stg_tile[0:rn, qt,
                                                         128 * m:128 * (m + 1)],
                                            identity=ident[0:rn, 0:rn])
                        nc.vector.tensor_copy(dstT[:, m, 128 * qt:128 * qt + rn],
                                              tp[:, 0:rn])

                KBLK = ((0, 512), (512, 512), (1024, 512), (1536, 64))

                def k_block(m, o, w):
                    kps = kqp.tile([128, 512], FP, tag="kq")
                    for c in range(2):
                        nc.tensor.matmul(out=kps[:, 0:w],
                                         lhsT=Wkp[:, c, 128 * m:128 * (m + 1)],
                                         rhs=efT[:, c, o:o + w],
                                         start=(c == 0), stop=(c == 1))
                    nc.vector.tensor_scalar_add(KT[:, m, o:o + w], kps[:, 0:w],
                                                bkT[:, m, :])

                def q_block(m):
                    qps = kqp.tile([128, 512], FP, tag="kq")
                    for c in range(2):
                        nc.tensor.matmul(out=qps[:, 0:NQ],
                                         lhsT=Wqp[:, c, 128 * m:128 * (m + 1)],
                                         rhs=efqT[:, c, :],
                                         start=(c == 0), stop=(c == 1))
                    nc.vector.tensor_scalar_add(QT[:, m, :], qps[:, 0:NQ],
                                                bqT[:, m, :])

                # transposes for K-block o, then the m=0 block (m=1 later)
                for bi, (o, w) in enumerate(KBLK):
                    for qt in range(o // 128, (o + w + 127) // 128):
                        rn = 128 if qt < 12 else 64
                        tp_tile(stg, qt, rn, efT)
                    k_block(0, o, w)
                for qt in range(4):
                    rn = 128 if qt < 3 else 16
                    tp_tile(stgq, qt, rn, efqT)
                q_block(0)
                for (o, w) in KBLK:
                    k_block(1, o, w)
                q_block(1)

            # =============== attention: 4 software-pipelined units ===============
            with tc.tile_pool(name="stp", bufs=2, space="PSUM") as stp_pool, \
                 tc.tile_pool(name="sums", bufs=2, space="PSUM") as sum_pool, \
                 tc.tile_pool(name="misc", bufs=2, space="PSUM") as misc_pool, \
                 tc.tile_pool(name="Epool", bufs=3) as E_pool, \
                 tc.tile_pool(name="Ppool", bufs=3) as P_pool, \
                 tc.tile_pool(name="rqpool", bufs=3) as rq_pool:

                units = [(qd, pair) for qd in range(2) for pair in range(2)]
                Etiles = {}
                Ptiles = {}
                upst = {}

                def emit_V():
                    for ck in range(NCH):
                        P = _chunk_p(ck)
                        vps = misc_pool.tile([128, 512], FP, tag="misc",
                                             name=f"vps{ck}")
                        for c in range(2):
                            nc.tensor.matmul(out=vps[0:P, 0:HID],
                                             lhsT=efT[:, c, CHUNK * ck:CHUNK * ck + P],
                                             rhs=Wvp[:, c, :], start=(c == 0),
                                             stop=False)
                        nc.tensor.matmul(out=vps[0:P, 0:HID],
                                         lhsT=onescol_bf[:, 0:P], rhs=bvrow[:],
                                         start=False, stop=True)
                        nc.vector.tensor_copy(Vt[0:P, ck, :], vps[0:P, 0:HID])

                def emit_S_chunk(u, ck, E, sums, iv, x0):
                    qd, pair = units[u]
                    P = _chunk_p(ck)
                    stp = stp_pool.tile([128, 2, 512], FP, tag="stp",
                                        name=f"stp{u}_{ck}")
                    for t in range(2):
                        hh = 2 * pair + t
                        nc.tensor.matmul(
                            out=stp[0:P, t, 0:NQ],
                            lhsT=KT[32 * hh:32 * hh + 32, qd,
                                    CHUNK * ck:CHUNK * ck + P],
                            rhs=QT[32 * hh:32 * hh + 32, qd, :],
                            tile_position=(32 * hh, 0),
                            start=True, stop=True)
                    nc.scalar.activation(out=E[0:P, :, ck, :],
                                         in_=stp[0:P, :, 0:NQ],
                                         func=AF.Exp, scale=float(ALPHA))
                    for t in range(2):
                        nc.tensor.matmul(
                            out=sums[64 * t:64 * t + 64, :],
                            lhsT=ind_sb[0:P, iv, x0:x0 + 64],
                            rhs=E[0:P, t, ck, :],
                            tile_position=(0, 64 * t),
                            start=False, stop=False,
                            skip_group_check=True)

                def emit_r(u, sums, tag, dram_off, nrow, exp_run, Pt, pck0, pckn):
                    # recip + bounce + expansion for one sums tile.
                    # dram r layout: [3 kb][exp_run] per t at dram_off(t).
                    rq = rq_pool.tile([128, NQ], BF, tag="rq", name="rq" + tag)
                    with nc.allow_low_precision(reason="softmax recip to bf16"):
                        nc.vector.reciprocal(rq[:], sums[:])
                    for t in range(2):
                        dst = bass.AP(tensor=r2, offset=dram_off(t),
                                      ap=[[400, nrow], [1, 400]])
                        nc.sync.dma_start(out=dst,
                                          in_=rq[64 * t:64 * t + nrow, :])
                    for t in range(2):
                        src = bass.AP(tensor=r2, offset=dram_off(t),
                                      ap=[[exp_run, 3], [0, 40], [1, exp_run]])
                        nc.gpsimd.dma_start(
                            out=Pt[:, t, pck0:pck0 + pckn, :].opt(), in_=src)

                def emit_A(u, split=False):
                    qd, pair = units[u]
                    E = E_pool.tile([CHUNK, 2, NCH, NQ], BF, tag="E",
                                    name=f"E{u}")
                    Etiles[u] = E
                    Pt = P_pool.tile([CHUNK, 2, NCH, NQ], BF, tag="P",
                                     name=f"P{u}")
                    Ptiles[u] = Pt
                    if not split:
                        sums = sum_pool.tile([128, NQ], FP, tag="sums",
                                             name=f"sums{u}")
                        nc.tensor.matmul(out=sums[:], lhsT=zrow[:],
                                         rhs=onesq[:], start=True, stop=False,
                                         skip_group_check=True)
                        for ck in range(NCH):
                            emit_S_chunk(u, ck, E, sums, 0, 13 - ck)
                        nc.tensor.matmul(out=sums[:], lhsT=zrow[:],
                                         rhs=onesq[:], start=False, stop=True,
                                         skip_group_check=True)
                        emit_r(u, sums, f"{u}",
                               lambda t: (u * 2 + t) * 3 * CKQ, 42, CKQ,
                               Pt, 0, NCH)
                    else:
                        # two ck-halves with the 7*kb+ck indicator
                        for half, (ck0, ckn) in enumerate(((0, 7), (7, 7))):
                            sums = sum_pool.tile([128, NQ], FP, tag="sums",
                                                 name=f"sums{u}_{half}")
                            nc.tensor.matmul(out=sums[:], lhsT=zrow[:],
                                             rhs=onesq[:], start=True,
                                             stop=False, skip_group_check=True)
                            for ck in range(ck0, ck0 + ckn):
                                emit_S_chunk(u, ck, E, sums, 1,
                                             13 + ck0 - ck)
                            nc.tensor.matmul(out=sums[:], lhsT=zrow[:],
                                             rhs=onesq[:], start=False,
                                             stop=True, skip_group_check=True)
                            emit_r(u, sums, f"{u}_{half}",
                                   lambda t: (u * 2 + t) * 3 * CKQ
                                   + half * 3 * 7 * NQ,
                                   21, 7 * NQ, Pt, ck0, ckn)

                def emit_B(u, cks=((0, 7), (7, 6), (13, 1))):
                    qd, pair = units[u]
                    E = Etiles.pop(u)
                    Pt = Ptiles.pop(u)
                    if pair == 0:
                        ups = misc_pool.tile([128, NQ], FP, tag="misc",
                                             name=f"ups{qd}")
                        upst[qd] = ups
                        nc.tensor.matmul(out=ups[:], lhsT=zrow[:], rhs=onesq[:],
                                         start=True, stop=False,
                                         skip_group_check=True)
                    ups = upst[qd]
                    for ck0, ckn in cks:
                        pe = min(ck0 + ckn, NCH - 1)
                        if pe > ck0:
                            nc.vector.tensor_mul(Pt[:, :, ck0:pe, :],
                                                 E[:, :, ck0:pe, :],
                                                 Pt[:, :, ck0:pe, :])
                        if ck0 + ckn == NCH:
                            nc.vector.tensor_mul(Pt[0:LAST_P, :, 13, :],
                                                 E[0:LAST_P, :, 13, :],
                                                 Pt[0:LAST_P, :, 13, :])
                        for ck in range(ck0, ck0 + ckn):
                            P = _chunk_p(ck)
                            for t in range(2):
                                hh = 2 * pair + t
                                nc.tensor.matmul(
                                    out=ups[32 * hh:32 * hh + 32, :],
                                    lhsT=Vt[0:P, ck, 128 * qd + 32 * hh:
                                            128 * qd + 32 * hh + 32],
                                    rhs=Pt[0:P, t, ck, :],
                                    tile_position=(0, 32 * hh),
                                    start=False, stop=False,
                                    skip_group_check=True)
                    if pair == 1:
                        nc.tensor.matmul(out=ups[:], lhsT=zrow[:], rhs=onesq[:],
                                         start=False, stop=True,
                                         skip_group_check=True)
                        nc.vector.tensor_copy(attT[:, qd, :], ups[:])
                        del upst[qd]

                emit_A(0)
                emit_V()
                emit_A(1)
                emit_B(0)
                emit_A(2)
                emit_B(1)
                emit_A(3, split=True)
                emit_B(2)
                emit_B(3, cks=((0, 7), (7, 4), (11, 2), (13, 1)))

            # ======== phase 3: gate + output projection ========
            with tc.tile_pool(name="p3psum", bufs=2, space="PSUM") as p3p:
                for m in range(2):
                    gps = p3p.tile([128, NQ], FP, tag="gps")
                    for c in range(2):
                        nc.tensor.matmul(out=gps[:],
                                         lhsT=Wtgb[:, c, 128 * m:128 * (m + 1)],
                                         rhs=attT[:, c, :], start=(c == 0),
                                         stop=False)
                    nc.tensor.matmul(out=gps[:],
                                     lhsT=bcat_sb[:, 3, 128 * m:128 * (m + 1)],
                                     rhs=onesq[:], start=False, stop=True)
                    nc.scalar.activation(out=gateT[:, m, :], in_=gps[:],
                                         func=AF.Exp, scale=-1.0)
                    nc.vector.tensor_scalar_add(gateT[:, m, :],
                                                gateT[:, m, :], 1.0)
                    with nc.allow_low_precision(reason="sigmoid recip bf16"):
                        nc.vector.reciprocal(gateT[:, m, :], gateT[:, m, :])
                    nc.vector.tensor_mul(gatedT[:, m, :], attT[:, m, :],
                                         gateT[:, m, :])
                for qc in range(4):
                    o0 = qc * 128
                    on = min(128, NQ - o0)
                    ops = p3p.tile([128, HID], FP, tag="ops")
                    for c in range(2):
                        nc.tensor.matmul(out=ops[0:on, :],
                                         lhsT=gatedT[:, c, o0:o0 + on],
                                         rhs=Wob[:, c, :], start=(c == 0),
                                         stop=False)
                    nc.tensor.matmul(out=ops[0:on, :],
                                     lhsT=ones128[:, 0:on],
                                     rhs=bcat_sb[:, 4, :],
                                     start=False, stop=True)
                    nc.vector.tensor_copy(osb[0:on, qc, :], ops[0:on, :])
                nc.sync.dma_start(
                    out=bass.AP(tensor=out, offset=0,
                                ap=[[HID, 128], [128 * HID, 3], [1, HID]]),
                    in_=osb[:, 0:3, :].opt())
                nc.sync.dma_start(out=out[384:400, :], in_=osb[0:16, 3, :])

    nc.finalize()
    return nc


def _get_compiled():
    global _COMPILED
    if _COMPILED is None:
        _COMPILED = _build_nc()
    return _COMPILED


def _numpy_reference(edge_features, edge_mask, condition, Wq, bq, Wk, bk, Wv, bv,
                     Wcp, bcp, Wcg, bcg, Wtb, btb, Wtg, btg, Wo, bo):
    def sig(x):
        return 1.0 / (1.0 + np.exp(-x))
    cond_proj = condition @ Wcp + bcp
    cond_gate = sig(condition @ Wcg + bcg)
    cf = edge_features * cond_gate[:, None, None, :] + cond_proj[:, None, None, :]
    Q = (cf @ Wq + bq).reshape(B, N, N, NH, HD)
    K = (cf @ Wk + bk).reshape(B, N, N, NH, HD)
    V = (cf @ Wv + bv).reshape(B, N, N, NH, HD)
    scores = np.einsum('bijhd,bklhd->bijklh', Q, K) / np.sqrt(HD).astype(np.float32)
    bias_in = np.concatenate(
        [cf, np.broadcast_to(condition[:, None, None, :], (B, N, N, CD))], axis=-1)
    bias = bias_in @ Wtb + btb
    scores = scores + bias[:, :, :, None, None, :]
    m = edge_mask[:, None, None, :, :, None] & edge_mask[:, :, :, None, None, None]
    scores = np.where(m, scores, -np.inf)
    mx = np.max(scores, axis=4, keepdims=True)
    mx = np.where(np.isfinite(mx), mx, 0.0)
    e = np.exp(scores - mx)
    attn = e / np.maximum(np.sum(e, axis=4, keepdims=True), 1e-30)
    attended = np.einsum('bijklh,bklhd->bijhd', attn, V).reshape(B, N, N, HID)
    gate = sig(attended @ Wtg + btg)
    return ((attended * gate) @ Wo + bo).astype(np.float32)


def _make_in_maps(ins):
    ef_full = np.ascontiguousarray(ins["edge_features"].astype(np.float32)
                                   .reshape(B, KL, HID))
    condition = ins["condition"].astype(np.float32)

    f32 = lambda k: ins[k].astype(np.float32)
    wcat = np.concatenate([f32("Wq"), f32("Wk"), f32("Wv"), f32("Wtg"),
                           f32("Wo"), f32("Wcp"), f32("Wcg")], axis=0)
    bcat = np.concatenate([f32(k).reshape(-1) for k in
                           ("bq", "bk", "bv", "btg", "bo", "bcp", "bcg")]
                          ).reshape(1, -1)
    shared = {
        "wcat": np.ascontiguousarray(wcat),
        "bcat": np.ascontiguousarray(bcat),
        "ind": _make_ind().reshape(CHUNK, 2 * 77),
    }
    in_maps = []
    for c in range(NCORES):
        b, s = c // 4, c % 4
        m = dict(shared)
        m["ef"] = np.ascontiguousarray(ef_full[b])
        m["efq"] = np.ascontiguousarray(ef_full[b, s * NQ:(s + 1) * NQ])
        m["cond"] = np.ascontiguousarray(condition[b:b + 1])
        in_maps.append(m)
    return in_maps


def kernel(**inputs):
    ins = {k: np.asarray(v) for k, v in inputs.items()}
    edge_mask = ins["edge_mask"]
    if not bool(edge_mask.all()):
        return _numpy_reference(
            ins["edge_features"].astype(np.float32), edge_mask.astype(bool),
            ins["condition"].astype(np.float32),
            *[ins[k].astype(np.float32) for k in
              ("Wq", "bq", "Wk", "bk", "Wv", "bv", "Wcp", "bcp", "Wcg", "bcg",
               "Wtb", "btb", "Wtg", "btg", "Wo", "bo")])

    in_maps = _make_in_maps(ins)
    from concourse.bass_utils import run_bass_kernel_spmd
    nc = _get_compiled()
    res = run_bass_kernel_spmd(nc, in_maps, core_ids=list(range(NCORES)))
    outs = [r["out"] for r in res.results]
    full = np.empty((B, KL, HID), np.float32)
    for c in range(NCORES):
        b, s = c // 4, c % 4
        full[b, s * NQ:(s + 1) * NQ] = outs[c]
    return full.reshape(B, N, N, HID)


if __name__ == "__main__":
    nc = _build_nc()
    print("built ok")


# revision 8
# speedup vs baseline: 2.0356x; 1.0867x over previous
"""Trainium2 Bass kernel for ConditionalTriangleAttention.

Reference computation (B=2, N=40, HID=256, NH=8, CD=128, HD=32):
  cf = edge_features * sigmoid(cond@Wcg+bcg) + (cond@Wcp+bcp)     (per batch)
  Q/K/V = cf @ W_{q,k,v} + b                                       [B,N,N,NH,HD]
  scores = einsum('bijhd,bklhd->bijklh', Q, K)/sqrt(HD) + bias     (bias const over k,l)
  attn = softmax over l;  attended = einsum('bijklh,bklhd->bijhd', attn, V)
  out = (attended * sigmoid(attended@Wtg+btg)) @ Wo + bo

With edge_mask all-ones (guaranteed by the input spec) the additive bias is
constant along the softmax axis and cancels, so Wtb/btb/edge_mask are no-ops.
A numpy fallback handles any other mask.

Sharding: 8 cores, each owns 400 query rows (b = core//4, i-rows slice) and
computes all heads for them end-to-end -- no collectives.

v3 pipeline per core: 4 units of (head-quad, head-pair), software-pipelined
A0 V A1 B0 A2 B1 A3 B2 B3.
  A(u): per kl-chunk S^T matmul (PE, double-buffered 2-bank PSUM) -> exp (ACT,
        the only ACT table used all kernel) -> per-k sums via indicator
        matmul (PE; k indexed 14*kb+ck so r lands DRAM-contiguous);
        reciprocal (DVE); r bounce to DRAM (SP); l-broadcast expansion as
        large-descriptor DMAs on the gpsimd queue.  The last unit splits its
        softmax tail into two ck-halves (second indicator, 7*kb+ck) to halve
        the end-of-pipe stall.
  B(u): P^T = E^T * R (DVE bf16 2x); U^T += V^T-contraction (PE).
Input DMAs are consolidated (one merged weight tensor, one merged bias row,
merged ef/efq loads); K^T is emitted interleaved with the ef transposes and
the attention pipeline starts after the m=0 half of K^T/Q^T.
"""

import os
import sys

for _p in ("/opt/trn_rl_repo", "/root/.axon_site/_ro/trn_rl_repo"):
    if os.path.isdir(_p) and _p not in sys.path:
        sys.path.insert(0, _p)

import numpy as np

B, N, HID, NH, CD = 2, 40, 256, 8, 128
HD = HID // NH            # 32
KL = N * N                # 1600
NQ = KL // 4              # 400 query rows per core
NCORES = 8
ALPHA = 1.0 / np.sqrt(np.float32(HD))

CHUNK = 120               # kl chunk: 3 k-groups of 40
NCH = 14                  # 13*120 + 40
LAST_P = KL - (NCH - 1) * CHUNK   # 40
CKQ = NCH * NQ            # 5600

_COMPILED = None


def _chunk_p(ck):
    return CHUNK if ck < NCH - 1 else LAST_P


def _make_ind():
    # indcat[:, 0, c]: 1 iff c == 14*(p//40) + 13   (full-unit sums layout)
    # indcat[:, 1, c]: 1 iff c ==  7*(p//40) + 13   (half-unit sums layout)
    ind = np.zeros((CHUNK, 2, 77), np.float32)
    for p in range(CHUNK):
        ind[p, 0, 14 * (p // 40) + 13] = 1.0
        ind[p, 1, 7 * (p // 40) + 13] = 1.0
    return ind


def _build_nc():
    import concourse.bass as bass
    import concourse.tile as tile
    from concourse import bacc, mybir
    from concourse.masks import make_identity

    FP = mybir.dt.float32
    BF = mybir.dt.bfloat16
    AF = mybir.ActivationFunctionType

    nc = bacc.Bacc(None, target_bir_lowering=False)

    ef = nc.dram_tensor("ef", [KL, HID], FP, kind="ExternalInput")
    efq = nc.dram_tensor("efq", [NQ, HID], FP, kind="ExternalInput")
    cond = nc.dram_tensor("cond", [1, CD], FP, kind="ExternalInput")
    ind = nc.dram_tensor("ind", [CHUNK, 2 * 77], FP, kind="ExternalInput")
    # wcat rows: Wq(256) Wk(256) Wv(256) Wtg(256) Wo(256) Wcp(128) Wcg(128)
    wcat = nc.dram_tensor("wcat", [12 * 128, HID], FP, kind="ExternalInput")
    # bcat rows: bq bk bv btg bo bcp bcg
    bcat = nc.dram_tensor("bcat", [1, 7 * HID], FP, kind="ExternalInput")
    out = nc.dram_tensor("out", [NQ, HID], FP, kind="ExternalOutput")

    # r bounce: slot u*2+t; full units use [3kb][CKQ]; the split unit (u=3)
    # uses [half][3kb][7*NQ].
    r2 = nc.dram_tensor("r2", [8, 3 * CKQ], BF, kind="Internal")

    W_OFF = {"Wq": 0, "Wk": 2, "Wv": 4, "Wtg": 6, "Wo": 8, "Wcp": 10, "Wcg": 11}
    B_OFF = {"bq": 0, "bk": 1, "bv": 2, "btg": 3, "bo": 4, "bcp": 5, "bcg": 6}

    with tile.TileContext(nc) as tc:
        with tc.tile_pool(name="persist", bufs=1) as sb:
            ident = sb.tile([128, 128], FP, tag="ident")
            make_identity(nc, ident)
            ones11 = sb.tile([1, 1], FP, tag="ones11")
            nc.vector.memset(ones11, 1.0)
            onesq = sb.tile([1, NQ], FP, tag="onesq")
            nc.vector.memset(onesq, 1.0)
            ones128 = sb.tile([1, 128], FP, tag="ones128")
            nc.vector.memset(ones128, 1.0)
            onescol_bf = sb.tile([1, CHUNK], BF, tag="onescol")
            nc.vector.memset(onescol_bf, 1.0)
            zrow = sb.tile([1, 128], FP, tag="zrow")
            nc.vector.memset(zrow, 0.0)

            ind_sb = sb.tile([CHUNK, 2, 77], BF, tag="ind_sb")

            # persistent bf16 operands
            efT = sb.tile([128, 2, KL], BF, tag="efT")
            efqT = sb.tile([128, 2, NQ], BF, tag="efqT")
            KT = sb.tile([128, 2, KL], BF, tag="KT")
            QT = sb.tile([128, 2, NQ], BF, tag="QT")
            Vt = sb.tile([128, NCH, HID], BF, tag="Vt")
            attT = sb.tile([128, 2, NQ], BF, tag="attT")
            gateT = sb.tile([128, 2, NQ], BF, tag="gateT")
            gatedT = sb.tile([128, 2, NQ], BF, tag="gatedT")
            # gated projection weights (bf16)
            Wqp = sb.tile([128, 2, HID], BF, tag="Wqp")
            Wkp = sb.tile([128, 2, HID], BF, tag="Wkp")
            Wvp = sb.tile([128, 2, HID], BF, tag="Wvp")
            Wtgb = sb.tile([128, 2, HID], BF, tag="Wtgb")
            Wob = sb.tile([128, 2, HID], BF, tag="Wob")
            # per-partition columns
            gT = sb.tile([128, 2, 1], FP, tag="gT")
            pT = sb.tile([128, 2, 1], FP, tag="pT")
            bqT = sb.tile([128, 2, 1], FP, tag="bqT")
            bkT = sb.tile([128, 2, 1], FP, tag="bkT")
            bvrow = sb.tile([1, HID], BF, tag="bvrow")
            bcat_sb = sb.tile([1, 7, HID], FP, tag="bcat_sb")
            osb = sb.tile([128, 4, HID], FP, tag="osb")

            # =============== phase 1: staging + cond + transposes + K/Q ========
            with tc.tile_pool(name="wstage", bufs=1) as ws, \
                 tc.tile_pool(name="p1psum", bufs=2, space="PSUM") as pp, \
                 tc.tile_pool(name="kqpsum", bufs=2, space="PSUM") as kqp, \
                 tc.tile_pool(name="tppsum", bufs=2, space="PSUM") as ppB:

                # ---- consolidated input loads ----
                cond_sb = ws.tile([1, CD], FP, tag="cond_sb")
                nc.scalar.dma_start(out=cond_sb[:], in_=cond[:])
                wst = ws.tile([128, 12, HID], FP, tag="wst")
                nc.scalar.dma_start(
                    out=wst[:, :, :].opt(),
                    in_=bass.AP(tensor=wcat, offset=0,
                                ap=[[HID, 128], [128 * HID, 12], [1, HID]]))
                nc.scalar.dma_start(out=bcat_sb[:, :, :].opt(),
                                    in_=bcat[:, :])
                ind_f = ws.tile([CHUNK, 2, 77], FP, tag="ind_f")
                nc.scalar.dma_start(out=ind_f[:, :, :].opt(),
                                    in_=ind[:, :])
                nc.vector.tensor_copy(ind_sb[:], ind_f[:])

                stg = ws.tile([128, 13, HID], FP, tag="stg")
                for h0, hn in ((0, 6), (6, 6)):
                    nc.sync.dma_start(
                        out=stg[:, h0:h0 + hn, :].opt(),
                        in_=bass.AP(tensor=ef, offset=h0 * 128 * HID,
                                    ap=[[HID, 128], [128 * HID, hn], [1, HID]]))
                nc.sync.dma_start(out=stg[0:64, 12, :], in_=ef[1536:1600, :])
                stgq = ws.tile([128, 4, HID], FP, tag="stgq")
                nc.sync.dma_start(
                    out=stgq[:, 0:3, :].opt(),
                    in_=bass.AP(tensor=efq, offset=0,
                                ap=[[HID, 128], [128 * HID, 3], [1, HID]]))
                nc.sync.dma_start(out=stgq[0:16, 3, :], in_=efq[384:400, :])

                def wslice(name, m, c0=0, cn=HID):
                    return wst[:, W_OFF[name] + m, c0:c0 + cn]

                def bslice(name, c0=0, cn=HID):
                    return bcat_sb[:, B_OFF[name], c0:c0 + cn]

                # ---- conditional gating columns ----
                ct_ps = pp.tile([128, 1], FP, tag="tiny")
                nc.tensor.matmul(out=ct_ps[:], lhsT=cond_sb[:], rhs=ones11[:],
                                 start=True, stop=True)
                condT = ws.tile([128, 1], FP, tag="condT")
                nc.vector.tensor_copy(condT[:], ct_ps[:])

                for m in range(2):
                    gp_ps = pp.tile([128, 1], FP, tag="tiny")
                    nc.tensor.matmul(out=gp_ps[:],
                                     lhsT=wslice("Wcg", 0, 128 * m, 128),
                                     rhs=condT[:], start=True, stop=False)
                    nc.tensor.matmul(out=gp_ps[:],
                                     lhsT=bslice("bcg", 128 * m, 128),
                                     rhs=ones11[:], start=False, stop=True)
                    # sigmoid via exp so ACT keeps the Exp table all kernel
                    nc.scalar.activation(out=gT[:, m, :], in_=gp_ps[:],
                                         func=AF.Exp, scale=-1.0)
                    nc.vector.tensor_scalar_add(gT[:, m, :], gT[:, m, :], 1.0)
                    nc.vector.reciprocal(gT[:, m, :], gT[:, m, :])

                    pp_ps = pp.tile([128, 1], FP, tag="tiny")
                    nc.tensor.matmul(out=pp_ps[:],
                                     lhsT=wslice("Wcp", 0, 128 * m, 128),
                                     rhs=condT[:], start=True, stop=False)
                    nc.tensor.matmul(out=pp_ps[:],
                                     lhsT=bslice("bcp", 128 * m, 128),
                                     rhs=ones11[:], start=False, stop=True)
                    nc.vector.tensor_copy(pT[:, m, :], pp_ps[:])

                # gated weights W' = diag(g) W  (bf16)
                for (wn, dst) in (("Wq", Wqp), ("Wk", Wkp), ("Wv", Wvp)):
                    for m in range(2):
                        nc.vector.tensor_scalar_mul(dst[:, m, :],
                                                    wslice(wn, m), gT[:, m, :])
                for (wn, dst) in (("Wtg", Wtgb), ("Wo", Wob)):
                    for m in range(2):
                        nc.vector.tensor_copy(dst[:, m, :], wslice(wn, m))

                # bias columns b' = (p @ W + b)^T  for q,k
                for (wn, bn, dst) in (("Wq", "bq", bqT), ("Wk", "bk", bkT)):
                    for m in range(2):
                        bps = pp.tile([128, 1], FP, tag="tiny")
                        for c in range(2):
                            nc.tensor.matmul(out=bps[:],
                                             lhsT=wslice(wn, c, 128 * m, 128),
                                             rhs=pT[:, c, :], start=(c == 0),
                                             stop=False)
                        nc.tensor.matmul(out=bps[:],
                                         lhsT=bslice(bn, 128 * m, 128),
                                         rhs=ones11[:], start=False, stop=True)
                        nc.vector.tensor_copy(dst[:, m, :], bps[:])
                # bv' as a row (used via rank-1 matmul into V)
                bvr_ps = pp.tile([1, HID], FP, tag="tiny")
                for c in range(2):
                    nc.tensor.matmul(out=bvr_ps[:], lhsT=pT[:, c, :],
                                     rhs=wslice("Wv", c), start=(c == 0),
                                     stop=False)
                nc.tensor.matmul(out=bvr_ps[:], lhsT=ones11[:],
                                 rhs=bslice("bv"), start=False, stop=True)
                nc.vector.tensor_copy(bvrow[:], bvr_ps[:])

                # ---- transposes interleaved with K^T blocks ----
                def tp_tile(stg_tile, qt, rn, dstT):
                    for m in range(2):
                        tp = ppB.tile([128, 128], FP, tag="tp")
                        nc.tensor.transpose(out=tp[:, 0:rn],
                                            in_=stg_tile[0:rn, qt,
                                                         128 * m:128 * (m + 1)],
                                            identity=ident[0:rn, 0:rn])
                        nc.vector.tensor_copy(dstT[:, m, 128 * qt:128 * qt + rn],
                                              tp[:, 0:rn])

                KBLK = ((0, 512), (512, 512), (1024, 512), (1536, 64))

                def k_block(m, o, w):
                    kps = kqp.tile([128, 512], FP, tag="kq")
                    for c in range(2):
                        nc.tensor.matmul(out=kps[:, 0:w],
                                         lhsT=Wkp[:, c, 128 * m:128 * (m + 1)],
                                         rhs=efT[:, c, o:o + w],
                                         start=(c == 0), stop=(c == 1))
                    nc.vector.tensor_scalar_add(KT[:, m, o:o + w], kps[:, 0:w],
                                                bkT[:, m, :])

                def q_block(m):
                    qps = kqp.tile([128, 512], FP, tag="kq")
                    for c in range(2):
                        nc.tensor.matmul(out=qps[:, 0:NQ],
                                         lhsT=Wqp[:, c, 128 * m:128 * (m + 1)],
                                         rhs=efqT[:, c, :],
                                         start=(c == 0), stop=(c == 1))
                    nc.vector.tensor_scalar_add(QT[:, m, :], qps[:, 0:NQ],
                                                bqT[:, m, :])

                # transposes for K-block o, then the m=0 block (m=1 later)
                for bi, (o, w) in enumerate(KBLK):
                    for qt in range(o // 128, (o + w + 127) // 128):
                        rn = 128 if qt < 12 else 64
                        tp_tile(stg, qt, rn, efT)
                    k_block(0, o, w)
                for qt in range(4):
                    rn = 128 if qt < 3 else 16
                    tp_tile(stgq, qt, rn, efqT)
                q_block(0)
                for (o, w) in KBLK:
                    k_block(1, o, w)
                q_block(1)

            # =============== attention: 4 software-pipelined units ===============
            with tc.tile_pool(name="stp", bufs=2, space="PSUM") as stp_pool, \
                 tc.tile_pool(name="sums", bufs=2, space="PSUM") as sum_pool, \
                 tc.tile_pool(name="misc", bufs=2, space="PSUM") as misc_pool, \
                 tc.tile_pool(name="Epool", bufs=3) as E_pool, \
                 tc.tile_pool(name="Ppool", bufs=3) as P_pool, \
                 tc.tile_pool(name="rqpool", bufs=3) as rq_pool:

                units = [(qd, pair) for qd in range(2) for pair in range(2)]
                Etiles = {}
                Ptiles = {}
                upst = {}

                def emit_V():
                    for ck in range(NCH):
                        P = _chunk_p(ck)
                        vps = misc_pool.tile([128, 512], FP, tag="misc",
                                             name=f"vps{ck}")
                        for c in range(2):
                            nc.tensor.matmul(out=vps[0:P, 0:HID],
                                             lhsT=efT[:, c, CHUNK * ck:CHUNK * ck + P],
                                             rhs=Wvp[:, c, :], start=(c == 0),
                                             stop=False)
                        nc.tensor.matmul(out=vps[0:P, 0:HID],
                                         lhsT=onescol_bf[:, 0:P], rhs=bvrow[:],
                                         start=False, stop=True)
                        nc.vector.tensor_copy(Vt[0:P, ck, :], vps[0:P, 0:HID])

                def emit_S_chunk(u, ck, E, sums, iv, x0, first, last):
                    qd, pair = units[u]
                    P = _chunk_p(ck)
                    stp = stp_pool.tile([128, 2, 512], FP, tag="stp",
                                        name=f"stp{u}_{ck}")
                    for t in range(2):
                        hh = 2 * pair + t
                        nc.tensor.matmul(
                            out=stp[0:P, t, 0:NQ],
                            lhsT=KT[32 * hh:32 * hh + 32, qd,
                                    CHUNK * ck:CHUNK * ck + P],
                            rhs=QT[32 * hh:32 * hh + 32, qd, :],
                            tile_position=(32 * hh, 0),
                            start=True, stop=True)
                    nc.scalar.activation(out=E[0:P, :, ck, :],
                                         in_=stp[0:P, :, 0:NQ],
                                         func=AF.Exp, scale=float(ALPHA))
                    for t in range(2):
                        nc.tensor.matmul(
                            out=sums[64 * t:64 * t + 64, :],
                            lhsT=ind_sb[0:P, iv, x0:x0 + 64],
                            rhs=E[0:P, t, ck, :],
                            tile_position=(0, 64 * t),
                            start=first, stop=last,
                            skip_group_check=True)

                def emit_r(u, sums, tag, dram_off, nrow, exp_run, Pt, pck0, pckn):
                    # recip + bounce + expansion for one sums tile.
                    # dram r layout: [3 kb][exp_run] per t at dram_off(t).
                    rq = rq_pool.tile([128, NQ], BF, tag="rq", name="rq" + tag)
                    with nc.allow_low_precision(reason="softmax recip to bf16"):
                        nc.vector.reciprocal(rq[:], sums[:])
                    for t in range(2):
                        dst = bass.AP(tensor=r2, offset=dram_off(t),
                                      ap=[[400, nrow], [1, 400]])
                        nc.sync.dma_start(out=dst,
                                          in_=rq[64 * t:64 * t + nrow, :])
                    for t in range(2):
                        src = bass.AP(tensor=r2, offset=dram_off(t),
                                      ap=[[exp_run, 3], [0, 40], [1, exp_run]])
                        nc.gpsimd.dma_start(
                            out=Pt[:, t, pck0:pck0 + pckn, :].opt(), in_=src)

                def emit_A(u, split=False):
                    qd, pair = units[u]
                    E = E_pool.tile([CHUNK, 2, NCH, NQ], BF, tag="E",
                                    name=f"E{u}")
                    Etiles[u] = E
                    Pt = P_pool.tile([CHUNK, 2, NCH, NQ], BF, tag="P",
                                     name=f"P{u}")
                    Ptiles[u] = Pt
                    if not split:
                        sums = sum_pool.tile([128, NQ], FP, tag="sums",
                                             name=f"sums{u}")
                        for ck in range(NCH):
                            emit_S_chunk(u, ck, E, sums, 0, 13 - ck,
                                         ck == 0, ck == NCH - 1)
                        emit_r(u, sums, f"{u}",
                               lambda t: (u * 2 + t) * 3 * CKQ, 42, CKQ,
                               Pt, 0, NCH)
                    else:
                        # two ck-halves with the 7*kb+ck indicator
                        for half, (ck0, ckn) in enumerate(((0, 7), (7, 7))):
                            sums = sum_pool.tile([128, NQ], FP, tag="sums",
                                                 name=f"sums{u}_{half}")
                            for ck in range(ck0, ck0 + ckn):
                                emit_S_chunk(u, ck, E, sums, 1,
                                             13 + ck0 - ck,
                                             ck == ck0, ck == ck0 + ckn - 1)
                            emit_r(u, sums, f"{u}_{half}",
                                   lambda t: (u * 2 + t) * 3 * CKQ
                                   + half * 3 * 7 * NQ,
                                   21, 7 * NQ, Pt, ck0, ckn)

                def emit_B(u, cks=((0, 7), (7, 6), (13, 1))):
                    qd, pair = units[u]
                    E = Etiles.pop(u)
                    Pt = Ptiles.pop(u)
                    if pair == 0:
                        ups = misc_pool.tile([128, NQ], FP, tag="misc",
                                             name=f"ups{qd}")
                        upst[qd] = ups
                    ups = upst[qd]
                    for ck0, ckn in cks:
                        pe = min(ck0 + ckn, NCH - 1)
                        if pe > ck0:
                            nc.vector.tensor_mul(Pt[:, :, ck0:pe, :],
                                                 E[:, :, ck0:pe, :],
                                                 Pt[:, :, ck0:pe, :])
                        if ck0 + ckn == NCH:
                            nc.vector.tensor_mul(Pt[0:LAST_P, :, 13, :],
                                                 E[0:LAST_P, :, 13, :],
                                                 Pt[0:LAST_P, :, 13, :])
                        for ck in range(ck0, ck0 + ckn):
                            P = _chunk_p(ck)
                            for t in range(2):
                                hh = 2 * pair + t
                                nc.tensor.matmul(
                                    out=ups[32 * hh:32 * hh + 32, :],
                                    lhsT=Vt[0:P, ck, 128 * qd + 32 * hh:
                                            128 * qd + 32 * hh + 32],
                                    rhs=Pt[0:P, t, ck, :],
                                    tile_position=(0, 32 * hh),
                                    start=(ck == 0), stop=(ck == NCH - 1),
                                    skip_group_check=True)
                    if pair == 1:
                        nc.vector.tensor_copy(attT[:, qd, :], ups[:])
                        del upst[qd]

                emit_A(0)
                emit_V()
                emit_A(1)
                emit_B(0)
                emit_A(2)
                emit_B(1)
                emit_A(3, split=True)
                emit_B(2)
                emit_B(3, cks=((0, 7), (7, 4), (11, 2), (13, 1)))

            # ======== phase 3: gate + output projection ========
            with tc.tile_pool(name="p3psum", bufs=7, space="PSUM") as p3p:
                gpss = []
                for m in range(2):
                    gps = p3p.tile([128, NQ], FP, tag="p3", name=f"gps{m}")
                    gpss.append(gps)
                    for c in range(2):
                        nc.tensor.matmul(out=gps[:],
                                         lhsT=Wtgb[:, c, 128 * m:128 * (m + 1)],
                                         rhs=attT[:, c, :], start=(c == 0),
                                         stop=False)
                    nc.tensor.matmul(out=gps[:],
                                     lhsT=bcat_sb[:, 3, 128 * m:128 * (m + 1)],
                                     rhs=onesq[:], start=False, stop=True)
                opss = []
                for m in range(2):
                    nc.scalar.activation(out=gateT[:, m, :], in_=gpss[m][:],
                                         func=AF.Sigmoid)
                    nc.vector.tensor_mul(gatedT[:, m, :], attT[:, m, :],
                                         gateT[:, m, :])
                    for qc in range(4):
                        o0 = qc * 128
                        on = min(128, NQ - o0)
                        if m == 0:
                            ops = p3p.tile([128, HID], FP, tag="p3",
                                           name=f"ops{qc}")
                            opss.append(ops)
                        ops = opss[qc]
                        nc.tensor.matmul(out=ops[0:on, :],
                                         lhsT=gatedT[:, m, o0:o0 + on],
                                         rhs=Wob[:, m, :], start=(m == 0),
                                         stop=False)
                        if m == 1:
                            nc.tensor.matmul(out=ops[0:on, :],
                                             lhsT=ones128[:, 0:on],
                                             rhs=bcat_sb[:, 4, :],
                                             start=False, stop=True)
                            nc.vector.tensor_copy(osb[0:on, qc, :],
                                                  ops[0:on, :])
                nc.sync.dma_start(
                    out=bass.AP(tensor=out, offset=0,
                                ap=[[HID, 128], [128 * HID, 3], [1, HID]]),
                    in_=osb[:, 0:3, :].opt())
                nc.sync.dma_start(out=out[384:400, :], in_=osb[0:16, 3, :])

    nc.finalize()
    return nc


def _get_compiled():
    global _COMPILED
    if _COMPILED is None:
        _COMPILED = _build_nc()
    return _COMPILED


def _numpy_reference(edge_features, edge_mask, condition, Wq, bq, Wk, bk, Wv, bv,
                     Wcp, bcp, Wcg, bcg, Wtb, btb, Wtg, btg, Wo, bo):
    def sig(x):
        return 1.0 / (1.0 + np.exp(-x))
    cond_proj = condition @ Wcp + bcp
    cond_gate = sig(condition @ Wcg + bcg)
    cf = edge_features * cond_gate[:, None, None, :] + cond_proj[:, None, None, :]
    Q = (cf @ Wq + bq).reshape(B, N, N, NH, HD)
    K = (cf @ Wk + bk).reshape(B, N, N, NH, HD)
    V = (cf @ Wv + bv).reshape(B, N, N, NH, HD)
    scores = np.einsum('bijhd,bklhd->bijklh', Q, K) / np.sqrt(HD).astype(np.float32)
    bias_in = np.concatenate(
        [cf, np.broadcast_to(condition[:, None, None, :], (B, N, N, CD))], axis=-1)
    bias = bias_in @ Wtb + btb
    scores = scores + bias[:, :, :, None, None, :]
    m = edge_mask[:, None, None, :, :, None] & edge_mask[:, :, :, None, None, None]
    scores = np.where(m, scores, -np.inf)
    mx = np.max(scores, axis=4, keepdims=True)
    mx = np.where(np.isfinite(mx), mx, 0.0)
    e = np.exp(scores - mx)
    attn = e / np.maximum(np.sum(e, axis=4, keepdims=True), 1e-30)
    attended = np.einsum('bijklh,bklhd->bijhd', attn, V).reshape(B, N, N, HID)
    gate = sig(attended @ Wtg + btg)
    return ((attended * gate) @ Wo + bo).astype(np.float32)


def _make_in_maps(ins):
    ef_full = np.ascontiguousarray(ins["edge_features"].astype(np.float32)
                                   .reshape(B, KL, HID))
    condition = ins["condition"].astype(np.float32)

    f32 = lambda k: ins[k].astype(np.float32)
    wcat = np.concatenate([f32("Wq"), f32("Wk"), f32("Wv"), f32("Wtg"),
                           f32("Wo"), f32("Wcp"), f32("Wcg")], axis=0)
    bcat = np.concatenate([f32(k).reshape(-1) for k in
                           ("bq", "bk", "bv", "btg", "bo", "bcp", "bcg")]
                          ).reshape(1, -1)
    shared = {
        "wcat": np.ascontiguousarray(wcat),
        "bcat": np.ascontiguousarray(bcat),
        "ind": _make_ind().reshape(CHUNK, 2 * 77),
    }
    in_maps = []
    for c in range(NCORES):
        b, s = c // 4, c % 4
        m = dict(shared)
        m["ef"] = np.ascontiguousarray(ef_full[b])
        m["efq"] = np.ascontiguousarray(ef_full[b, s * NQ:(s + 1) * NQ])
        m["cond"] = np.ascontiguousarray(condition[b:b + 1])
        in_maps.append(m)
    return in_maps


def kernel(**inputs):
    ins = {k: np.asarray(v) for k, v in inputs.items()}
    edge_mask = ins["edge_mask"]
    if not bool(edge_mask.all()):
        return _numpy_reference(
            ins["edge_features"].astype(np.float32), edge_mask.astype(bool),
            ins["condition"].astype(np.float32),
            *[ins[k].astype(np.float32) for k in
              ("Wq", "bq", "Wk", "bk", "Wv", "bv", "Wcp", "bcp", "Wcg", "bcg",
               "Wtb", "btb", "Wtg", "btg", "Wo", "bo")])

    in_maps = _make_in_maps(ins)
    from concourse.bass_utils import run_bass_kernel_spmd
    nc = _get_compiled()
    res = run_bass_kernel_spmd(nc, in_maps, core_ids=list(range(NCORES)))
    outs = [r["out"] for r in res.results]
    full = np.empty((B, KL, HID), np.float32)
    for c in range(NCORES):
        b, s = c // 4, c % 4
        full[b, s * NQ:(s + 1) * NQ] = outs[c]
    return full.reshape(B, N, N, HID)


if __name__ == "__main__":
    nc = _build_nc()
    print("built ok")


# revision 10
# speedup vs baseline: 2.1104x; 1.0368x over previous
"""Trainium2 Bass kernel for ConditionalTriangleAttention.

Reference computation (B=2, N=40, HID=256, NH=8, CD=128, HD=32):
  cf = edge_features * sigmoid(cond@Wcg+bcg) + (cond@Wcp+bcp)     (per batch)
  Q/K/V = cf @ W_{q,k,v} + b                                       [B,N,N,NH,HD]
  scores = einsum('bijhd,bklhd->bijklh', Q, K)/sqrt(HD) + bias     (bias const over k,l)
  attn = softmax over l;  attended = einsum('bijklh,bklhd->bijhd', attn, V)
  out = (attended * sigmoid(attended@Wtg+btg)) @ Wo + bo

With edge_mask all-ones (guaranteed by the input spec) the additive bias is
constant along the softmax axis and cancels, so Wtb/btb/edge_mask are no-ops.
A numpy fallback handles any other mask.

Sharding: 8 cores, each owns 400 query rows (b = core//4, i-rows slice) and
computes all heads for them end-to-end -- no collectives.

v3 pipeline per core: 4 units of (head-quad, head-pair), software-pipelined
A0 V A1 B0 A2 B1 A3 B2 B3.
  A(u): per kl-chunk S^T matmul (PE, double-buffered 2-bank PSUM) -> exp (ACT,
        the only ACT table used all kernel) -> per-k sums via indicator
        matmul (PE; k indexed 14*kb+ck so r lands DRAM-contiguous);
        reciprocal (DVE); r bounce to DRAM (SP); l-broadcast expansion as
        large-descriptor DMAs on the gpsimd queue.  The last unit splits its
        softmax tail into two ck-halves (second indicator, 7*kb+ck) to halve
        the end-of-pipe stall.
  B(u): P^T = E^T * R (DVE bf16 2x); U^T += V^T-contraction (PE).
Input DMAs are consolidated (one merged weight tensor, one merged bias row,
merged ef/efq loads); K^T is emitted interleaved with the ef transposes and
the attention pipeline starts after the m=0 half of K^T/Q^T.
"""

import os
import sys

for _p in ("/opt/trn_rl_repo", "/root/.axon_site/_ro/trn_rl_repo"):
    if os.path.isdir(_p) and _p not in sys.path:
        sys.path.insert(0, _p)

import numpy as np

B, N, HID, NH, CD = 2, 40, 256, 8, 128
HD = HID // NH            # 32
KL = N * N                # 1600
NQ = KL // 4              # 400 query rows per core
NCORES = 8
ALPHA = 1.0 / np.sqrt(np.float32(HD))

CHUNK = 120               # kl chunk: 3 k-groups of 40
NCH = 14                  # 13*120 + 40
LAST_P = KL - (NCH - 1) * CHUNK   # 40
CKQ = NCH * NQ            # 5600

_COMPILED = None


def _chunk_p(ck):
    return CHUNK if ck < NCH - 1 else LAST_P


def _make_ind():
    # indcat[:, 0, c]: 1 iff c == 14*(p//40) + 13   (full-unit sums layout)
    # indcat[:, 1, c]: 1 iff c ==  7*(p//40) + 13   (half-unit sums layout)
    ind = np.zeros((CHUNK, 2, 77), np.float32)
    for p in range(CHUNK):
        ind[p, 0, 14 * (p // 40) + 13] = 1.0
        ind[p, 1, 7 * (p // 40) + 13] = 1.0
    return ind


def _build_nc():
    import concourse.bass as bass
    import concourse.tile as tile
    from concourse import bacc, mybir
    from concourse.masks import make_identity

    FP = mybir.dt.float32
    BF = mybir.dt.bfloat16
    AF = mybir.ActivationFunctionType

    nc = bacc.Bacc(None, target_bir_lowering=False)

    ef = nc.dram_tensor("ef", [KL, HID], FP, kind="ExternalInput")
    efq = nc.dram_tensor("efq", [NQ, HID], FP, kind="ExternalInput")
    cond = nc.dram_tensor("cond", [1, CD], FP, kind="ExternalInput")
    ind = nc.dram_tensor("ind", [CHUNK, 2 * 77], FP, kind="ExternalInput")
    # wcat rows: Wq(256) Wk(256) Wv(256) Wtg(256) Wo(256) Wcp(128) Wcg(128)
    wcat = nc.dram_tensor("wcat", [12 * 128, HID], FP, kind="ExternalInput")
    # bcat rows: bq bk bv btg bo bcp bcg
    bcat = nc.dram_tensor("bcat", [1, 7 * HID], FP, kind="ExternalInput")
    out = nc.dram_tensor("out", [NQ, HID], FP, kind="ExternalOutput")

    # r bounce: slot u*2+t; full units use [3kb][CKQ]; the split unit (u=3)
    # uses [half][3kb][7*NQ].
    r2 = nc.dram_tensor("r2", [8, 3 * CKQ], BF, kind="Internal")

    W_OFF = {"Wq": 0, "Wk": 2, "Wv": 4, "Wtg": 6, "Wo": 8, "Wcp": 10, "Wcg": 11}
    B_OFF = {"bq": 0, "bk": 1, "bv": 2, "btg": 3, "bo": 4, "bcp": 5, "bcg": 6}

    with tile.TileContext(nc) as tc:
        with tc.tile_pool(name="persist", bufs=1) as sb:
            identb = sb.tile([128, 128], BF, tag="identb")
            make_identity(nc, identb)
            ones11 = sb.tile([1, 1], FP, tag="ones11")
            nc.vector.memset(ones11, 1.0)
            onesq = sb.tile([1, NQ], FP, tag="onesq")
            nc.vector.memset(onesq, 1.0)
            ones128 = sb.tile([1, 128], FP, tag="ones128")
            nc.vector.memset(ones128, 1.0)

            ind_sb = sb.tile([CHUNK, 2, 77], BF, tag="ind_sb")

            # persistent bf16 operands
            efT = sb.tile([128, 2, KL], BF, tag="efT")
            efqT = sb.tile([128, 2, NQ], BF, tag="efqT")
            KT = sb.tile([128, 2, KL], BF, tag="KT")
            QT = sb.tile([128, 2, NQ], BF, tag="QT")
            Vt = sb.tile([128, NCH, HID], BF, tag="Vt")
            attT = sb.tile([128, 2, NQ], BF, tag="attT")
            gateT = sb.tile([128, 2, NQ], BF, tag="gateT")
            gatedT = sb.tile([128, 2, NQ], BF, tag="gatedT")
            # gated projection weights (bf16)
            Wqp = sb.tile([128, 2, HID], BF, tag="Wqp")
            Wkp = sb.tile([128, 2, HID], BF, tag="Wkp")
            Wvp = sb.tile([128, 2, HID], BF, tag="Wvp")
            Wtgb = sb.tile([128, 2, HID], BF, tag="Wtgb")
            Wob = sb.tile([128, 2, HID], BF, tag="Wob")
            # per-partition columns
            gT = sb.tile([128, 2, 1], FP, tag="gT")
            pT = sb.tile([128, 2, 1], FP, tag="pT")
            bqT = sb.tile([128, 2, 1], FP, tag="bqT")
            bkT = sb.tile([128, 2, 1], FP, tag="bkT")
            bv40T = sb.tile([128, 2, 1], FP, tag="bv40T")
            bcat_sb = sb.tile([1, 7, HID], FP, tag="bcat_sb")
            osb = sb.tile([128, 4, HID], FP, tag="osb")

            # =============== phase 1: staging + cond + transposes + K/Q ========
            with tc.tile_pool(name="wstage", bufs=1) as ws, \
                 tc.tile_pool(name="p1psum", bufs=2, space="PSUM") as pp, \
                 tc.tile_pool(name="kqpsum", bufs=2, space="PSUM") as kqp, \
                 tc.tile_pool(name="tppsum", bufs=2, space="PSUM") as ppB:

                # ---- consolidated input loads ----
                cond_sb = ws.tile([1, CD], FP, tag="cond_sb")
                nc.scalar.dma_start(out=cond_sb[:], in_=cond[:])
                wst = ws.tile([128, 12, HID], FP, tag="wst")
                nc.scalar.dma_start(
                    out=wst[:, :, :].opt(),
                    in_=bass.AP(tensor=wcat, offset=0,
                                ap=[[HID, 128], [128 * HID, 12], [1, HID]]))
                nc.gpsimd.dma_start(out=bcat_sb[:, :, :].opt(),
                                    in_=bcat[:, :])
                nc.gpsimd.dma_start(out=ind_sb[:, :, :].opt(), in_=ind[:, :])

                stg = ws.tile([128, 13, HID], BF, tag="stg")
                for h0, hn in ((0, 4), (4, 4), (8, 4)):
                    nc.gpsimd.dma_start(
                        out=stg[:, h0:h0 + hn, :].opt(),
                        in_=bass.AP(tensor=ef, offset=h0 * 128 * HID,
                                    ap=[[HID, 128], [128 * HID, hn], [1, HID]]))
                nc.gpsimd.dma_start(out=stg[0:64, 12, :],
                                    in_=ef[1536:1600, :])
                stgq = ws.tile([128, 4, HID], BF, tag="stgq")
                nc.gpsimd.dma_start(
                    out=stgq[:, 0:3, :].opt(),
                    in_=bass.AP(tensor=efq, offset=0,
                                ap=[[HID, 128], [128 * HID, 3], [1, HID]]))
                nc.gpsimd.dma_start(out=stgq[0:16, 3, :],
                                    in_=efq[384:400, :])

                def wslice(name, m, c0=0, cn=HID):
                    return wst[:, W_OFF[name] + m, c0:c0 + cn]

                def bslice(name, c0=0, cn=HID):
                    return bcat_sb[:, B_OFF[name], c0:c0 + cn]

                # ---- conditional gating columns ----
                ct_ps = pp.tile([128, 1], FP, tag="tiny")
                nc.tensor.matmul(out=ct_ps[:], lhsT=cond_sb[:], rhs=ones11[:],
                                 start=True, stop=True)
                condT = ws.tile([128, 1], FP, tag="condT")
                nc.vector.tensor_copy(condT[:], ct_ps[:])

                for m in range(2):
                    gp_ps = pp.tile([128, 1], FP, tag="tiny")
                    nc.tensor.matmul(out=gp_ps[:],
                                     lhsT=wslice("Wcg", 0, 128 * m, 128),
                                     rhs=condT[:], start=True, stop=False)
                    nc.tensor.matmul(out=gp_ps[:],
                                     lhsT=bslice("bcg", 128 * m, 128),
                                     rhs=ones11[:], start=False, stop=True)
                    # sigmoid via exp so ACT keeps the Exp table all kernel
                    nc.scalar.activation(out=gT[:, m, :], in_=gp_ps[:],
                                         func=AF.Exp, scale=-1.0)
                    nc.vector.tensor_scalar_add(gT[:, m, :], gT[:, m, :], 1.0)
                    nc.vector.reciprocal(gT[:, m, :], gT[:, m, :])

                    pp_ps = pp.tile([128, 1], FP, tag="tiny")
                    nc.tensor.matmul(out=pp_ps[:],
                                     lhsT=wslice("Wcp", 0, 128 * m, 128),
                                     rhs=condT[:], start=True, stop=False)
                    nc.tensor.matmul(out=pp_ps[:],
                                     lhsT=bslice("bcp", 128 * m, 128),
                                     rhs=ones11[:], start=False, stop=True)
                    nc.vector.tensor_copy(pT[:, m, :], pp_ps[:])

                # gated weights W' = diag(g) W  (bf16)
                for (wn, dst) in (("Wq", Wqp), ("Wk", Wkp), ("Wv", Wvp)):
                    for m in range(2):
                        nc.vector.tensor_scalar_mul(dst[:, m, :],
                                                    wslice(wn, m), gT[:, m, :])
                for (wn, dst) in (("Wtg", Wtgb), ("Wo", Wob)):
                    for m in range(2):
                        nc.vector.tensor_copy(dst[:, m, :], wslice(wn, m))

                # bias columns b' = (p @ W + b)^T  for q,k
                for (wn, bn, dst) in (("Wq", "bq", bqT), ("Wk", "bk", bkT)):
                    for m in range(2):
                        bps = pp.tile([128, 1], FP, tag="tiny")
                        for c in range(2):
                            nc.tensor.matmul(out=bps[:],
                                             lhsT=wslice(wn, c, 128 * m, 128),
                                             rhs=pT[:, c, :], start=(c == 0),
                                             stop=False)
                        nc.tensor.matmul(out=bps[:],
                                         lhsT=bslice(bn, 128 * m, 128),
                                         rhs=ones11[:], start=False, stop=True)
                        nc.vector.tensor_copy(dst[:, m, :], bps[:])
                # bv' as a column scaled by 40: softmax rows sum to 1 over
                # each of the 40 k-groups, so the V bias contributes exactly
                # 40*bv' to attended -- fold it into the attT evacuation.
                for m in range(2):
                    bps = pp.tile([128, 1], FP, tag="tiny")
                    for c in range(2):
                        nc.tensor.matmul(out=bps[:],
                                         lhsT=wslice("Wv", c, 128 * m, 128),
                                         rhs=pT[:, c, :], start=(c == 0),
                                         stop=False)
                    nc.tensor.matmul(out=bps[:],
                                     lhsT=bslice("bv", 128 * m, 128),
                                     rhs=ones11[:], start=False, stop=True)
                    nc.vector.tensor_scalar_mul(bv40T[:, m, :], bps[:], 40.0)

                # ---- transposes interleaved with K^T blocks ----
                def tp_tile(stg_tile, qt, rn, dstT):
                    for m in range(2):
                        tp = ppB.tile([128, 128], BF, tag="tp")
                        nc.tensor.transpose(out=tp[:, 0:rn],
                                            in_=stg_tile[0:rn, qt,
                                                         128 * m:128 * (m + 1)],
                                            identity=identb[0:rn, 0:rn])
                        nc.scalar.activation(
                            out=dstT[:, m, 128 * qt:128 * qt + rn],
                            in_=tp[:, 0:rn], func=AF.Identity)

                KBLK = ((0, 512), (512, 512), (1024, 512), (1536, 64))

                def k_block(m, o, w):
                    kps = kqp.tile([128, 512], FP, tag="kq")
                    for c in range(2):
                        nc.tensor.matmul(out=kps[:, 0:w],
                                         lhsT=Wkp[:, c, 128 * m:128 * (m + 1)],
                                         rhs=efT[:, c, o:o + w],
                                         start=(c == 0), stop=(c == 1))
                    nc.vector.tensor_scalar_add(KT[:, m, o:o + w], kps[:, 0:w],
                                                bkT[:, m, :])

                def q_block(m):
                    qps = kqp.tile([128, 512], FP, tag="kq")
                    for c in range(2):
                        nc.tensor.matmul(out=qps[:, 0:NQ],
                                         lhsT=Wqp[:, c, 128 * m:128 * (m + 1)],
                                         rhs=efqT[:, c, :],
                                         start=(c == 0), stop=(c == 1))
                    nc.vector.tensor_scalar_add(QT[:, m, :], qps[:, 0:NQ],
                                                bqT[:, m, :])

                # transposes for K-block o, then the m=0 block (m=1 later)
                for bi, (o, w) in enumerate(KBLK):
                    for qt in range(o // 128, (o + w + 127) // 128):
                        rn = 128 if qt < 12 else 64
                        tp_tile(stg, qt, rn, efT)
                    k_block(0, o, w)
                for qt in range(4):
                    rn = 128 if qt < 3 else 16
                    tp_tile(stgq, qt, rn, efqT)
                q_block(0)
                for (o, w) in KBLK:
                    k_block(1, o, w)
                q_block(1)

            # =============== attention: 4 software-pipelined units ===============
            with tc.tile_pool(name="stp", bufs=2, space="PSUM") as stp_pool, \
                 tc.tile_pool(name="sums", bufs=2, space="PSUM") as sum_pool, \
                 tc.tile_pool(name="misc", bufs=2, space="PSUM") as misc_pool, \
                 tc.tile_pool(name="Epool", bufs=3) as E_pool, \
                 tc.tile_pool(name="Ppool", bufs=3) as P_pool, \
                 tc.tile_pool(name="rqpool", bufs=3) as rq_pool:

                units = [(qd, pair) for qd in range(2) for pair in range(2)]
                Etiles = {}
                Ptiles = {}
                upst = {}

                def emit_V():
                    for ck in range(NCH):
                        P = _chunk_p(ck)
                        vps = misc_pool.tile([128, 512], FP, tag="misc",
                                             name=f"vps{ck}")
                        for c in range(2):
                            nc.tensor.matmul(out=vps[0:P, 0:HID],
                                             lhsT=efT[:, c, CHUNK * ck:CHUNK * ck + P],
                                             rhs=Wvp[:, c, :], start=(c == 0),
                                             stop=(c == 1))
                        nc.vector.tensor_copy(Vt[0:P, ck, :], vps[0:P, 0:HID])

                def emit_S_chunk(u, ck, E, sums, iv, x0, first, last):
                    qd, pair = units[u]
                    P = _chunk_p(ck)
                    stp = stp_pool.tile([128, 2, 512], FP, tag="stp",
                                        name=f"stp{u}_{ck}")
                    for t in range(2):
                        hh = 2 * pair + t
                        nc.tensor.matmul(
                            out=stp[0:P, t, 0:NQ],
                            lhsT=KT[32 * hh:32 * hh + 32, qd,
                                    CHUNK * ck:CHUNK * ck + P],
                            rhs=QT[32 * hh:32 * hh + 32, qd, :],
                            tile_position=(32 * hh, 0),
                            start=True, stop=True)
                    nc.scalar.activation(out=E[0:P, :, ck, :],
                                         in_=stp[0:P, :, 0:NQ],
                                         func=AF.Exp, scale=float(ALPHA))
                    for t in range(2):
                        nc.tensor.matmul(
                            out=sums[64 * t:64 * t + 64, :],
                            lhsT=ind_sb[0:P, iv, x0:x0 + 64],
                            rhs=E[0:P, t, ck, :],
                            tile_position=(0, 64 * t),
                            start=first, stop=last,
                            skip_group_check=True)

                def emit_r(u, sums, tag, dram_off, nrow, exp_run, Pt, pck0, pckn):
                    # recip + bounce + expansion for one sums tile.
                    # dram r layout: [3 kb][exp_run] per t at dram_off(t).
                    rq = rq_pool.tile([128, NQ], BF, tag="rq", name="rq" + tag)
                    with nc.allow_low_precision(reason="softmax recip to bf16"):
                        nc.vector.reciprocal(rq[:], sums[:])
                    for t in range(2):
                        dst = bass.AP(tensor=r2, offset=dram_off(t),
                                      ap=[[400, nrow], [1, 400]])
                        nc.sync.dma_start(out=dst,
                                          in_=rq[64 * t:64 * t + nrow, :])
                    for t in range(2):
                        src = bass.AP(tensor=r2, offset=dram_off(t),
                                      ap=[[exp_run, 3], [0, 40], [1, exp_run]])
                        nc.gpsimd.dma_start(
                            out=Pt[:, t, pck0:pck0 + pckn, :].opt(), in_=src)

                def emit_A(u, split=False):
                    qd, pair = units[u]
                    E = E_pool.tile([CHUNK, 2, NCH, NQ], BF, tag="E",
                                    name=f"E{u}")
                    Etiles[u] = E
                    Pt = P_pool.tile([CHUNK, 2, NCH, NQ], BF, tag="P",
                                     name=f"P{u}")
                    Ptiles[u] = Pt
                    if not split:
                        sums = sum_pool.tile([128, NQ], FP, tag="sums",
                                             name=f"sums{u}")
                        for ck in range(NCH):
                            emit_S_chunk(u, ck, E, sums, 0, 13 - ck,
                                         ck == 0, ck == NCH - 1)
                        emit_r(u, sums, f"{u}",
                               lambda t: (u * 2 + t) * 3 * CKQ, 42, CKQ,
                               Pt, 0, NCH)
                    else:
                        # two ck-halves with the 7*kb+ck indicator
                        for half, (ck0, ckn) in enumerate(((0, 7), (7, 7))):
                            sums = sum_pool.tile([128, NQ], FP, tag="sums",
                                                 name=f"sums{u}_{half}")
                            for ck in range(ck0, ck0 + ckn):
                                emit_S_chunk(u, ck, E, sums, 1,
                                             13 + ck0 - ck,
                                             ck == ck0, ck == ck0 + ckn - 1)
                            emit_r(u, sums, f"{u}_{half}",
                                   lambda t: (u * 2 + t) * 3 * CKQ
                                   + half * 3 * 7 * NQ,
                                   21, 7 * NQ, Pt, ck0, ckn)

                def emit_B(u, cks=((0, 7), (7, 6), (13, 1))):
                    qd, pair = units[u]
                    E = Etiles.pop(u)
                    Pt = Ptiles.pop(u)
                    if pair == 0:
                        ups = misc_pool.tile([128, NQ], FP, tag="misc",
                                             name=f"ups{qd}")
                        upst[qd] = ups
                    ups = upst[qd]
                    for ck0, ckn in cks:
                        pe = min(ck0 + ckn, NCH - 1)
                        if pe > ck0:
                            nc.vector.tensor_mul(Pt[:, :, ck0:pe, :],
                                                 E[:, :, ck0:pe, :],
                                                 Pt[:, :, ck0:pe, :])
                        if ck0 + ckn == NCH:
                            nc.vector.tensor_mul(Pt[0:LAST_P, :, 13, :],
                                                 E[0:LAST_P, :, 13, :],
                                                 Pt[0:LAST_P, :, 13, :])
                        for ck in range(ck0, ck0 + ckn):
                            P = _chunk_p(ck)
                            for t in range(2):
                                hh = 2 * pair + t
                                nc.tensor.matmul(
                                    out=ups[32 * hh:32 * hh + 32, :],
                                    lhsT=Vt[0:P, ck, 128 * qd + 32 * hh:
                                            128 * qd + 32 * hh + 32],
                                    rhs=Pt[0:P, t, ck, :],
                                    tile_position=(0, 32 * hh),
                                    start=(ck == 0), stop=(ck == NCH - 1),
                                    skip_group_check=True)
                    if pair == 1:
                        nc.vector.tensor_scalar_add(attT[:, qd, :], ups[:],
                                                    bv40T[:, qd, :])
                        del upst[qd]

                emit_A(0)
                emit_V()
                emit_A(1)
                emit_B(0)
                emit_A(2)
                emit_B(1)
                emit_A(3, split=True)
                emit_B(2)
                emit_B(3, cks=((0, 7), (7, 4), (11, 2), (13, 1)))

            # ======== phase 3: gate + output projection ========
            with tc.tile_pool(name="p3psum", bufs=7, space="PSUM") as p3p:
                gpss = []
                for m in range(2):
                    gps = p3p.tile([128, NQ], FP, tag="p3", name=f"gps{m}")
                    gpss.append(gps)
                    for c in range(2):
                        nc.tensor.matmul(out=gps[:],
                                         lhsT=Wtgb[:, c, 128 * m:128 * (m + 1)],
                                         rhs=attT[:, c, :], start=(c == 0),
                                         stop=False)
                    nc.tensor.matmul(out=gps[:],
                                     lhsT=bcat_sb[:, 3, 128 * m:128 * (m + 1)],
                                     rhs=onesq[:], start=False, stop=True)
                opss = []
                for m in range(2):
                    nc.scalar.activation(out=gateT[:, m, :], in_=gpss[m][:],
                                         func=AF.Sigmoid)
                    nc.vector.tensor_mul(gatedT[:, m, :], attT[:, m, :],
                                         gateT[:, m, :])
                    for qc in range(4):
                        o0 = qc * 128
                        on = min(128, NQ - o0)
                        if m == 0:
                            ops = p3p.tile([128, HID], FP, tag="p3",
                                           name=f"ops{qc}")
                            opss.append(ops)
                        ops = opss[qc]
                        nc.tensor.matmul(out=ops[0:on, :],
                                         lhsT=gatedT[:, m, o0:o0 + on],
                                         rhs=Wob[:, m, :], start=(m == 0),
                                         stop=False)
                        if m == 1:
                            nc.tensor.matmul(out=ops[0:on, :],
                                             lhsT=ones128[:, 0:on],
                                             rhs=bcat_sb[:, 4, :],
                                             start=False, stop=True)
                            nc.vector.tensor_copy(osb[0:on, qc, :],
                                                  ops[0:on, :])
                nc.sync.dma_start(
                    out=bass.AP(tensor=out, offset=0,
                                ap=[[HID, 128], [128 * HID, 3], [1, HID]]),
                    in_=osb[:, 0:3, :].opt())
                nc.sync.dma_start(out=out[384:400, :], in_=osb[0:16, 3, :])

    nc.finalize()
    return nc


def _get_compiled():
    global _COMPILED
    if _COMPILED is None:
        _COMPILED = _build_nc()
    return _COMPILED


def _numpy_reference(edge_features, edge_mask, condition, Wq, bq, Wk, bk, Wv, bv,
                     Wcp, bcp, Wcg, bcg, Wtb, btb, Wtg, btg, Wo, bo):
    def sig(x):
        return 1.0 / (1.0 + np.exp(-x))
    cond_proj = condition @ Wcp + bcp
    cond_gate = sig(condition @ Wcg + bcg)
    cf = edge_features * cond_gate[:, None, None, :] + cond_proj[:, None, None, :]
    Q = (cf @ Wq + bq).reshape(B, N, N, NH, HD)
    K = (cf @ Wk + bk).reshape(B, N, N, NH, HD)
    V = (cf @ Wv + bv).reshape(B, N, N, NH, HD)
    scores = np.einsum('bijhd,bklhd->bijklh', Q, K) / np.sqrt(HD).astype(np.float32)
    bias_in = np.concatenate(
        [cf, np.broadcast_to(condition[:, None, None, :], (B, N, N, CD))], axis=-1)
    bias = bias_in @ Wtb + btb
    scores = scores + bias[:, :, :, None, None, :]
    m = edge_mask[:, None, None, :, :, None] & edge_mask[:, :, :, None, None, None]
    scores = np.where(m, scores, -np.inf)
    mx = np.max(scores, axis=4, keepdims=True)
    mx = np.where(np.isfinite(mx), mx, 0.0)
    e = np.exp(scores - mx)
    attn = e / np.maximum(np.sum(e, axis=4, keepdims=True), 1e-30)
    attended = np.einsum('bijklh,bklhd->bijhd', attn, V).reshape(B, N, N, HID)
    gate = sig(attended @ Wtg + btg)
    return ((attended * gate) @ Wo + bo).astype(np.float32)


def _make_in_maps(ins):
    ef_full = np.ascontiguousarray(ins["edge_features"].astype(np.float32)
                                   .reshape(B, KL, HID))
    condition = ins["condition"].astype(np.float32)

    f32 = lambda k: ins[k].astype(np.float32)
    wcat = np.concatenate([f32("Wq"), f32("Wk"), f32("Wv"), f32("Wtg"),
                           f32("Wo"), f32("Wcp"), f32("Wcg")], axis=0)
    bcat = np.concatenate([f32(k).reshape(-1) for k in
                           ("bq", "bk", "bv", "btg", "bo", "bcp", "bcg")]
                          ).reshape(1, -1)
    shared = {
        "wcat": np.ascontiguousarray(wcat),
        "bcat": np.ascontiguousarray(bcat),
        "ind": _make_ind().reshape(CHUNK, 2 * 77),
    }
    in_maps = []
    for c in range(NCORES):
        b, s = c // 4, c % 4
        m = dict(shared)
        m["ef"] = np.ascontiguousarray(ef_full[b])
        m["efq"] = np.ascontiguousarray(ef_full[b, s * NQ:(s + 1) * NQ])
        m["cond"] = np.ascontiguousarray(condition[b:b + 1])
        in_maps.append(m)
    return in_maps


def kernel(**inputs):
    ins = {k: np.asarray(v) for k, v in inputs.items()}
    edge_mask = ins["edge_mask"]
    if not bool(edge_mask.all()):
        return _numpy_reference(
            ins["edge_features"].astype(np.float32), edge_mask.astype(bool),
            ins["condition"].astype(np.float32),
            *[ins[k].astype(np.float32) for k in
              ("Wq", "bq", "Wk", "bk", "Wv", "bv", "Wcp", "bcp", "Wcg", "bcg",
               "Wtb", "btb", "Wtg", "btg", "Wo", "bo")])

    in_maps = _make_in_maps(ins)
    from concourse.bass_utils import run_bass_kernel_spmd
    nc = _get_compiled()
    res = run_bass_kernel_spmd(nc, in_maps, core_ids=list(range(NCORES)))
    outs = [r["out"] for r in res.results]
    full = np.empty((B, KL, HID), np.float32)
    for c in range(NCORES):
        b, s = c // 4, c % 4
        full[b, s * NQ:(s + 1) * NQ] = outs[c]
    return full.reshape(B, N, N, HID)


if __name__ == "__main__":
    nc = _build_nc()
    print("built ok")
